# revision 1
# baseline (speedup 1.0000x reference)
"""Trainium2 Bass kernel for nn_CustomABlock (MDTA transformer block).

Per-core layout: one batch image [C=256, N=4096(=64x64)] per NeuronCore,
data-parallel over B=8 across 8 cores, all params replicated.

Engine plan (per core):
  PE   : qkv matmul (f32r), 2 dwconv taps (diag matmul), q/k transposes,
         gram (attn logits), attn@v, proj, mlp1, mlp2
  DVE  : 6 dwconv taps (scalar_tensor_tensor FMA, bf16), residual adds,
         reciprocals, row-max reduces, x1 bf16 copy
  ACT  : PSUM drains, l2norm squares (accum), exp (softmax), gelu+bias
  GPSIMD: 1 dwconv tap, identity build
"""

import numpy as np
import ml_dtypes

BF16 = ml_dtypes.bfloat16

C = 256          # dim
N = 4096         # 64*64
H = W = 64
NH = 8           # heads
CH = 32          # channels per head
HID = 307        # mlp hidden
NB_QKV = 6       # qkv channel blocks of 128
NT = 8           # n tiles of 512
TS = 512

# tap index t = (dy+1)*3 + (dx+1)
PE_TAPS = [(0, 0), (-1, 0), (1, 0), (0, -1), (0, 1)]  # PE diag matmuls into PSUM
MERGE_TAP = (1, 1)                  # DVE STT: tap + PSUM drain in one op
DVE_TAPS = [(-1, -1), (-1, 1), (1, -1)]   # DVE tensor_scalar + tensor_tensor

_CACHE = {}


def _build_bass():
    import concourse.bass as bass
    from concourse import bacc
    from concourse import mybir
    from concourse.tile import TileContext
    from concourse.masks import make_identity

    dt = mybir.dt
    f32 = dt.float32
    f32r = dt.float32r
    bf16 = dt.bfloat16
    AF = mybir.ActivationFunctionType
    OP = mybir.AluOpType

    nc = bacc.Bacc("TRN2")

    # ---- DRAM I/O (per-core) ----
    x_d = nc.dram_tensor("x", [128, 2, N], f32, kind="ExternalInput")
    xb_d = nc.dram_tensor("xb", [128, 2, N], bf16, kind="ExternalInput")
    wqkv_d = nc.dram_tensor("wqkvT", [128, 2, 3 * C], bf16, kind="ExternalInput")
    wdiag_d = nc.dram_tensor("wdiag", [128, len(PE_TAPS), NB_QKV, 128], bf16,
                             kind="ExternalInput")
    wdw_d = nc.dram_tensor("wdw", [128, NB_QKV * 9 * 2], f32, kind="ExternalInput")
    wproj_d = nc.dram_tensor("wprojT", [128, 2, C], bf16, kind="ExternalInput")
    wm1_d = nc.dram_tensor("wm1T", [128, 2, HID], bf16, kind="ExternalInput")
    wm2_d = nc.dram_tensor("wm2T", [128, 3, C], bf16, kind="ExternalInput")
    b1_d = nc.dram_tensor("b1", [128, 3], f32, kind="ExternalInput")
    b2_d = nc.dram_tensor("b2", [128, 2], f32, kind="ExternalInput")
    tv_d = nc.dram_tensor("tempvec", [128, 2], f32, kind="ExternalInput")
    out_d = nc.dram_tensor("out", [128, 2, N], f32, kind="ExternalOutput")

    with TileContext(nc) as tc:
        with (
            tc.tile_pool(name="wpool", bufs=1) as wpool,
            tc.tile_pool(name="xpool", bufs=1) as xpool,
            tc.tile_pool(name="qkvp", bufs=3) as qkvp,       # qkv_s blocks / ys reuse
            tc.tile_pool(name="dwqk", bufs=3) as dwqk_p,     # dw q/k blocks / attn_s reuse
            tc.tile_pool(name="dwv", bufs=2) as dwv_p,       # dw v blocks / x1b reuse
            tc.tile_pool(name="qt", bufs=1) as qt_p,
            tc.tile_pool(name="scr", bufs=2) as scr_p,
            tc.tile_pool(name="small", bufs=10) as small_p,
            tc.tile_pool(name="apool", bufs=2) as a_p,
            tc.tile_pool(name="pbig", bufs=2, space="PSUM") as pbig,
            tc.tile_pool(name="pdw", bufs=2, space="PSUM") as pdw,
            tc.tile_pool(name="psml", bufs=2, space="PSUM") as psml,
        ):
            # ---- load weights & x (critical path first) ----
            xs = xpool.tile([128, 2, N], f32)
            xr = xpool.tile([128, 2, N], bf16)
            wqkv_s = wpool.tile([128, 2, 3 * C], bf16)
            for kb in range(2):
                nc.sync.dma_start(out=wqkv_s[:, kb, :], in_=wqkv_d[:, kb, :])
                nc.sync.dma_start(out=xr[:, kb, :], in_=xb_d[:, kb, :])
            wdiag_s = wpool.tile([128, len(PE_TAPS), NB_QKV, 128], bf16)
            nc.sync.dma_start(out=wdiag_s, in_=wdiag_d[:, :, :, :])
            wdw_s = wpool.tile([128, NB_QKV * 9 * 2], f32)
            nc.sync.dma_start(out=wdw_s, in_=wdw_d[:, :])
            for kb in range(2):
                nc.sync.dma_start(out=xs[:, kb, :], in_=x_d[:, kb, :])
            wproj_s = wpool.tile([128, 2, C], bf16)
            nc.sync.dma_start(out=wproj_s, in_=wproj_d[:, :, :])
            wm1_s = wpool.tile([128, 2, HID], bf16)
            nc.sync.dma_start(out=wm1_s, in_=wm1_d[:, :, :])
            wm2_s = wpool.tile([128, 3, C], bf16)
            nc.sync.dma_start(out=wm2_s, in_=wm2_d[:, :, :])
            b1_s = wpool.tile([128, 3], f32)
            nc.sync.dma_start(out=b1_s, in_=b1_d[:, :])
            b2_s = wpool.tile([128, 2], f32)
            nc.sync.dma_start(out=b2_s, in_=b2_d[:, :])
            tv_s = wpool.tile([128, 2], f32)
            nc.sync.dma_start(out=tv_s, in_=tv_d[:, :])

            ident = wpool.tile([128, 128], bf16)
            make_identity(nc, ident)

            # ---- per-block pipeline ----
            dw_tiles = [None] * NB_QKV
            qT_s = qt_p.tile([128, 32, C], bf16, tag="qT")
            kT_s = qt_p.tile([128, 32, C], bf16, tag="kT")
            attn_s = [dwqk_p.tile([128, N], bf16, tag="dwqk", name=f"attn{g}")
                      for g in range(2)]
            rs_v = [None, None]
            At_v = [None, None]

            def do_block(ob):
                # qkv = W_qkv @ x  (bf16) -> PSUM [128,1024] -> bf16 SBUF
                qkv_t = qkvp.tile([128, N], bf16, tag="qkv", name=f"qkv{ob}")
                for t in range(4):
                    ps = pbig.tile([128, 1024], f32, tag="pbig", name="ps")
                    for h in range(2):
                        for kb in range(2):
                            nc.tensor.matmul(
                                ps[:, h * TS:(h + 1) * TS],
                                lhsT=wqkv_s[:, kb, ob * 128:(ob + 1) * 128],
                                rhs=xr[:, kb, t * 1024 + h * TS:
                                       t * 1024 + (h + 1) * TS],
                                start=(kb == 0), stop=(kb == 1),
                            )
                    nc.scalar.copy(out=qkv_t[:, t * 1024:(t + 1) * 1024], in_=ps)

                # dwconv: 5 PE diag taps (flat shifts) -> PSUM[128,512]
                dw_t = (dwqk_p if ob < 4 else dwv_p).tile(
                    [128, N], bf16, tag=("dwqk" if ob < 4 else "dwv"),
                    name=f"dw{ob}")
                dw_tiles[ob] = dw_t
                dw3 = dw_t.rearrange("p (y x) -> p y x", y=H)
                qk3 = qkv_t.rearrange("p (y x) -> p y x", y=H)
                dym, dxm = MERGE_TAP
                wm = wdw_s[:, ob * 9 + (dym + 1) * 3 + dxm + 1:
                           ob * 9 + (dym + 1) * 3 + dxm + 2]
                w01 = wdw_s[:, 54 + ob * 9 + 5:54 + ob * 9 + 6]
                for t8 in range(8):
                    pd = pdw.tile([128, TS], f32, tag="pdw", name="pd")
                    pd3 = pd.rearrange("p (y x) -> p y x", y=8)
                    c0 = t8 * TS
                    ops = []
                    for ti, (dy, dx) in enumerate(PE_TAPS):
                        s = dy * 64 + dx
                        a = max(c0, -s)
                        b = min(c0 + TS, N - max(0, s))
                        if a < b:
                            ops.append((ti, s, a, b))
                    for j, (ti, s, a, b) in enumerate(ops):
                        nc.tensor.matmul(
                            pd[:, a - c0:b - c0],
                            lhsT=wdiag_s[:, ti, ob, :],
                            rhs=qkv_t[:, a + s:b + s],
                            start=(j == 0), stop=(j == len(ops) - 1),
                        )
                    yt = t8 * 8
                    # merge tap (1,1): dw = w*qkv[y+1,x+1] + psum (drains)
                    ya, yb = yt, min(yt + 8, 63)
                    nc.vector.scalar_tensor_tensor(
                        out=dw3[:, ya:yb, 0:63],
                        in0=qk3[:, ya + 1:yb + 1, 1:64],
                        scalar=wm,
                        in1=pd3[:, 0:yb - yt, 0:63],
                        op0=OP.mult, op1=OP.add,
                    )
                    # x=63 col: drain PSUM minus tap(0,1) row-wrap
                    nc.vector.scalar_tensor_tensor(
                        out=dw3[:, yt:yb, 63:64],
                        in0=qk3[:, yt + 1:yb + 1, 0:1],
                        scalar=w01, in1=pd3[:, 0:yb - yt, 63:64],
                        op0=OP.mult, op1=OP.add,
                    )
                    if t8 == 7:
                        nc.scalar.copy(out=dw3[:, 63:64, :],
                                       in_=pd3[:, 7:8, :])
                # x=0 col: subtract tap(0,-1) row-wrap (whole block, in place)
                w0m = wdw_s[:, 54 + ob * 9 + 3:54 + ob * 9 + 4]
                nc.vector.scalar_tensor_tensor(
                    out=dw3[:, 1:64, 0:1], in0=qk3[:, 0:63, 63:64],
                    scalar=w0m, in1=dw3[:, 1:64, 0:1],
                    op0=OP.mult, op1=OP.add,
                )

                # 3 corner taps: tensor_scalar (4x) + tensor_tensor (2x)
                for (dy, dx) in DVE_TAPS:
                    ti = (dy + 1) * 3 + (dx + 1)
                    w_ap = wdw_s[:, ob * 9 + ti:ob * 9 + ti + 1]
                    y0, y1 = max(0, -dy), 64 - max(0, dy)
                    x0, x1 = max(0, -dx), 64 - max(0, dx)
                    sc_t = scr_p.tile([128, N], bf16, tag="sqscr",
                                      name=f"scr{ob}_{ti}")
                    sc3 = sc_t.rearrange("p (y x) -> p y x", y=H)
                    nc.vector.tensor_scalar_mul(
                        sc3[:, y0:y1, x0:x1],
                        qk3[:, y0 + dy:y1 + dy, x0 + dx:x1 + dx], w_ap)
                    nc.vector.tensor_tensor(
                        out=dw3[:, y0:y1, x0:x1], in0=dw3[:, y0:y1, x0:x1],
                        in1=sc3[:, y0:y1, x0:x1], op=OP.add)

                # q/k blocks: l2 norm row-scale then transpose to [n, c]
                if ob < 4:
                    sq = scr_p.tile([128, N], bf16, tag="sqscr")
                    ssq = small_p.tile([128, 1], f32, tag="ssq")
                    nc.scalar.activation(out=sq, in_=dw_t, func=AF.Square,
                                         accum_out=ssq)
                    nrm = small_p.tile([128, 1], f32, tag="nrm")
                    nc.scalar.sqrt(nrm, ssq)
                    rn = small_p.tile([128, 1], f32, tag="rn")
                    nc.vector.reciprocal(rn, nrm)
                    if ob < 2:   # q rows: fold temperature in
                        sc = small_p.tile([128, 1], f32, tag="sc")
                        nc.vector.tensor_mul(sc, rn, tv_s[:, ob:ob + 1])
                        rowscale = sc
                    else:
                        rowscale = rn
                    nc.vector.tensor_scalar_mul(dw_t, dw_t, rowscale)
                    dst = qT_s if ob < 2 else kT_s
                    cof = (ob % 2) * 128
                    for g in range(8):
                        tp_t = psml.tile([128, 512], bf16, tag="tp")
                        for i in range(4):
                            nb = g * 4 + i
                            nc.tensor.transpose(
                                tp_t[:, i * 128:(i + 1) * 128],
                                dw_t[:, nb * 128:(nb + 1) * 128], ident)
                        nc.scalar.copy(
                            out=dst[:, g * 4:g * 4 + 4, cof:cof + 128],
                            in_=tp_t.rearrange("p (a b) -> p a b", a=4))

            def do_gram(g):
                pg = psml.tile([128, 128], f32, tag="tp")
                co = g * 128
                for nb in range(32):
                    nc.tensor.matmul(
                        pg,
                        lhsT=qT_s[:, nb, co:co + 128],
                        rhs=kT_s[:, nb, co:co + 128],
                        start=(nb == 0), stop=(nb == 31),
                    )
                A_t = a_p.tile([128, 128], bf16, tag="A")
                nc.vector.memset(A_t, 0.0)
                mx = small_p.tile([128, 1], f32, tag="mx")
                sm = small_p.tile([128, 1], f32, tag="sm")
                for h in range(4):
                    r0, r1 = h * 32, h * 32 + 32
                    nc.vector.tensor_reduce(
                        out=mx[r0:r1, :], in_=pg[r0:r1, r0:r1],
                        axis=mybir.AxisListType.X, op=OP.max)
                nc.vector.tensor_scalar_mul(mx, mx, -1.0)
                for h in range(4):
                    r0, r1 = h * 32, h * 32 + 32
                    nc.scalar.activation(
                        out=A_t[r0:r1, r0:r1], in_=pg[r0:r1, r0:r1],
                        func=AF.Exp, bias=mx[r0:r1, :],
                        accum_out=sm[r0:r1, :])
                rs = small_p.tile([128, 1], f32, tag="rs")
                nc.vector.reciprocal(rs, sm)
                rs_v[g] = rs
                pa = psml.tile([128, 128], bf16, tag="tp")
                nc.tensor.transpose(pa, A_t, ident)
                At = a_p.tile([128, 128], bf16, tag="At")
                nc.scalar.copy(out=At, in_=pa)
                At_v[g] = At

            def do_av(g):
                for t in range(4):
                    pv = pbig.tile([128, 1024], f32, tag="pbig", name="pv")
                    for h in range(2):
                        nc.tensor.matmul(
                            pv[:, h * TS:(h + 1) * TS], lhsT=At_v[g],
                            rhs=dw_tiles[4 + g][:, t * 1024 + h * TS:
                                                t * 1024 + (h + 1) * TS],
                            start=True, stop=True)
                    nc.scalar.mul(attn_s[g][:, t * 1024:(t + 1) * 1024],
                                  pv, rs_v[g])

            do_block(0)
            do_block(2)
            do_gram(0)
            do_block(1)
            do_block(3)
            do_gram(1)
            do_block(4)
            do_av(0)
            do_block(5)
            do_av(1)

            # ---- streamed tail: proj+resid1 / mlp1 / mlp2+resid2+DMA per tile ----
            x1b = [dwv_p.tile([128, N], bf16, tag="dwv", name=f"x1b{i}")
                   for i in range(2)]
            ys = [qkvp.tile([128, N], bf16, tag="qkv", name=f"ys{i}")
                  for i in range(3)]
            for t in range(4):
                sl = slice(t * 1024, (t + 1) * 1024)
                for ob in range(2):
                    pp = pbig.tile([128, 1024], f32, tag="pbig", name="pp")
                    for h in range(2):
                        for kb in range(2):
                            nc.tensor.matmul(
                                pp[:, h * TS:(h + 1) * TS],
                                lhsT=wproj_s[:, kb, ob * 128:(ob + 1) * 128],
                                rhs=attn_s[kb][:, t * 1024 + h * TS:
                                               t * 1024 + (h + 1) * TS],
                                start=(kb == 0), stop=(kb == 1))
                    nc.vector.tensor_tensor(
                        out=xs[:, ob, sl], in0=xs[:, ob, sl], in1=pp, op=OP.add)
                    nc.vector.tensor_copy(out=x1b[ob][:, sl], in_=xs[:, ob, sl])
                for mb in range(3):
                    rows = 128 if mb < 2 else HID - 256
                    pm = pbig.tile([128, 1024], f32, tag="pbig", name="pm")
                    for h in range(2):
                        for kb in range(2):
                            nc.tensor.matmul(
                                pm[:rows, h * TS:(h + 1) * TS],
                                lhsT=wm1_s[:, kb, mb * 128:mb * 128 + rows],
                                rhs=x1b[kb][:, t * 1024 + h * TS:
                                            t * 1024 + (h + 1) * TS],
                                start=(kb == 0), stop=(kb == 1))
                    nc.scalar.activation(
                        out=ys[mb][:rows, sl],
                        in_=pm[:rows, :], func=AF.Gelu_apprx_tanh,
                        bias=b1_s[:rows, mb:mb + 1])
                for ob in range(2):
                    pm2 = pbig.tile([128, 1024], f32, tag="pbig", name="pm2")
                    for h in range(2):
                        for kb in range(3):
                            rows = 128 if kb < 2 else HID - 256
                            nc.tensor.matmul(
                                pm2[:, h * TS:(h + 1) * TS],
                                lhsT=wm2_s[:rows, kb, ob * 128:(ob + 1) * 128],
                                rhs=ys[kb][:rows, t * 1024 + h * TS:
                                           t * 1024 + (h + 1) * TS],
                                start=(kb == 0), stop=(kb == 2))
                    nc.vector.scalar_tensor_tensor(
                        out=xs[:, ob, sl], in0=pm2,
                        scalar=b2_s[:, ob:ob + 1], in1=xs[:, ob, sl],
                        op0=OP.add, op1=OP.add)
                    nc.sync.dma_start(out=out_d[:, ob, sl], in_=xs[:, ob, sl])

    return nc


def _prep_shared(w_qkv, w_dw, temperature, w_proj, w_mlp1, b_mlp1, w_mlp2, b_mlp2):
    f32 = np.float32
    shared = {}
    shared["wqkvT"] = np.ascontiguousarray(
        w_qkv.T.reshape(2, 128, 3 * C).transpose(1, 0, 2)).astype(BF16)
    wd = np.zeros((128, len(PE_TAPS), NB_QKV, 128), BF16)
    for ti, (dy, dx) in enumerate(PE_TAPS):
        for cb in range(NB_QKV):
            w = w_dw[cb * 128:(cb + 1) * 128, 0, dy + 1, dx + 1].astype(f32)
            wd[:, ti, cb, :] = np.diag(w).astype(BF16)
    shared["wdiag"] = wd
    wt = np.zeros((128, NB_QKV * 9 * 2), f32)
    for cb in range(NB_QKV):
        for t in range(9):
            wt[:, cb * 9 + t] = w_dw[cb * 128:(cb + 1) * 128, 0, t // 3, t % 3]
    wt[:, 54:] = -wt[:, :54]
    shared["wdw"] = wt
    shared["wprojT"] = np.ascontiguousarray(
        w_proj.T.reshape(2, 128, C).transpose(1, 0, 2)).astype(BF16)
    shared["wm1T"] = np.ascontiguousarray(
        w_mlp1.T.reshape(2, 128, HID).transpose(1, 0, 2)).astype(BF16)
    w2 = np.zeros((384, C), f32)
    w2[:HID] = w_mlp2.T
    shared["wm2T"] = np.ascontiguousarray(
        w2.reshape(3, 128, C).transpose(1, 0, 2)).astype(BF16)
    b1 = np.zeros((384,), f32)
    b1[:HID] = b_mlp1
    shared["b1"] = np.ascontiguousarray(b1.reshape(3, 128).T)
    shared["b2"] = np.ascontiguousarray(b_mlp2.astype(f32).reshape(2, 128).T)
    t = temperature.reshape(NH).astype(f32)
    tv = np.zeros((128, 2), f32)
    for g in range(2):
        tv[:, g] = np.repeat(t[g * 4:(g + 1) * 4], 32)
    shared["tempvec"] = tv
    return shared


def kernel(x, w_qkv, w_dw, temperature, w_proj, w_mlp1, b_mlp1, w_mlp2, b_mlp2,
           _trace=False):
    from concourse.bass_utils import run_bass_kernel_spmd

    if "nc" not in _CACHE:
        nc = _build_bass()
        nc.finalize()
        _CACHE["nc"] = nc
    nc = _CACHE["nc"]

    x = np.asarray(x, np.float32)
    B = x.shape[0]
    shared = _prep_shared(
        np.asarray(w_qkv, np.float32), np.asarray(w_dw, np.float32),
        np.asarray(temperature, np.float32), np.asarray(w_proj, np.float32),
        np.asarray(w_mlp1, np.float32), np.asarray(b_mlp1, np.float32),
        np.asarray(w_mlp2, np.float32), np.asarray(b_mlp2, np.float32))

    in_maps = []
    for i in range(B):
        m = dict(shared)
        xi = np.ascontiguousarray(x[i].reshape(2, 128, N).transpose(1, 0, 2))
        m["x"] = xi
        m["xb"] = xi.astype(BF16)
        in_maps.append(m)

    res = run_bass_kernel_spmd(nc, in_maps, core_ids=list(range(B)),
                               trace=_trace)
    outs = np.stack([
        r["out"].transpose(1, 0, 2).reshape(C, H, W) for r in res.results
    ])
    if _trace:
        _CACHE["last_exec_ns"] = res.exec_time_ns
        _CACHE["last_profile"] = res.profile_json
    return outs



# revision 29
# speedup vs baseline: 1.4682x; 1.4682x over previous
"""Trainium2 Bass kernel for nn_CustomABlock (MDTA transformer block).

Per-core layout: one batch image [C=256, N=4096(=64x64)] per NeuronCore,
data-parallel over B=8 across 8 cores, all params replicated.

dwconv 3x3 tap split (s = 64*dy + dx, flat shift over zero-padded qkv):
  q/k blocks: 6 PE diag-matmul taps (corners + (0,+-1)); DVE: (-1,0) folded
    into the PSUM-drain STT, (0,0) and (1,0) as TS(4x)+TT(2x) pairs.
  v blocks: 8 PE taps (adds (0,0),(1,0)); DVE only drains + fixes —
    frees DVE in the attention/tail transition.
  6 x-wrap column fixes per block (negated weights), all DVE work split
  per half-block so downstream av/tail unlock early.
Residuals in bf16 (x loaded bf16 only); softmax without the row-max pass
(|logit| <= t by Cauchy-Schwarz on normalized q,k); temperature/|q| folded
into the EXP scale; |k| scale applied once per k block; all four
sqrt/recips batched into one op each to minimize ACT table loads.
"""

import numpy as np
import ml_dtypes

BF16 = ml_dtypes.bfloat16

C = 256          # dim
N = 4096         # 64*64
H = W = 64
NH = 8           # heads
HID = 307        # mlp hidden
NB_QKV = 6       # qkv channel blocks of 128
TS = 512
PAD = 66         # zero pad elems each side of qkv (covers |s| <= 65, even)

# PE taps: flat full-range diag matmuls over the zero-padded qkv buffer.
PE_TAPS = [(0, -1), (0, 1), (-1, -1), (-1, 1), (1, -1), (1, 1)]
V_EXTRA = [(0, 0), (1, 0)]           # extra PE taps for v blocks
# x-wrap column fixes for PE taps: (tap, out_y0, out_y1, out_x, in_dy, in_x)
#   dw3[:, y0:y1, ox] -= w * qk3[:, y0+idy:y1+idy, ix]
FIXES = [
    ((0, -1), 1, 64, 0, -1, 63),
    ((-1, -1), 2, 64, 0, -2, 63),
    ((1, -1), 0, 64, 0, 0, 63),
    ((0, 1), 0, 63, 63, 1, 0),
    ((-1, 1), 0, 64, 63, 0, 0),
    ((1, 1), 0, 62, 63, 2, 0),
]

_CACHE = {}


def _build_bass():
    import concourse.bass as bass
    from concourse import bacc
    from concourse import mybir
    from concourse.tile import TileContext
    from concourse.masks import make_identity

    dt = mybir.dt
    f32 = dt.float32
    bf16 = dt.bfloat16
    AF = mybir.ActivationFunctionType
    OP = mybir.AluOpType

    nc = bacc.Bacc("TRN2")

    # ---- DRAM I/O (per-core) ----
    xb_d = nc.dram_tensor("xb", [128, 2, N], bf16, kind="ExternalInput")
    wqkv_d = nc.dram_tensor("wqkvT", [128, 2, 3 * C], bf16, kind="ExternalInput")
    wdiag_d = nc.dram_tensor("wdiag", [128, 8, NB_QKV, 128], bf16,
                             kind="ExternalInput")
    # wdw[:, ob, t]: 9 taps fp32 per block; wdw[:, 6+ob, t]: negated
    wdw_d = nc.dram_tensor("wdw", [128, 12, 9], f32, kind="ExternalInput")
    wproj_d = nc.dram_tensor("wprojT", [128, 2, C], bf16, kind="ExternalInput")
    wm1_d = nc.dram_tensor("wm1T", [128, 2, HID], bf16, kind="ExternalInput")
    wm2_d = nc.dram_tensor("wm2T", [128, 3, C], bf16, kind="ExternalInput")
    b1_d = nc.dram_tensor("b1", [128, 3], f32, kind="ExternalInput")
    b2_d = nc.dram_tensor("b2", [128, 2], f32, kind="ExternalInput")
    tv_d = nc.dram_tensor("tempvec", [128, 2], f32, kind="ExternalInput")
    out_d = nc.dram_tensor("out", [128, 2, N], bf16, kind="ExternalOutput")

    with TileContext(nc) as tc:
        with (
            tc.tile_pool(name="wpool", bufs=1) as wpool,
            tc.tile_pool(name="xpool", bufs=1) as xpool,
            tc.tile_pool(name="qkvp", bufs=3) as qkvp,      # qkv blocks / ys reuse
            tc.tile_pool(name="dwqk", bufs=4) as dwqk_p,    # dw q/k blocks / attn_s
            tc.tile_pool(name="dwv", bufs=3) as dwv_p,      # dw v blocks / x1b reuse
            tc.tile_pool(name="qt", bufs=1) as qt_p,
            tc.tile_pool(name="scr", bufs=2) as scr_p,
            tc.tile_pool(name="small", bufs=12) as small_p,
            tc.tile_pool(name="apool", bufs=2) as a_p,
            tc.tile_pool(name="pq", bufs=3, space="PSUM") as pq,      # [128,512] f32
            tc.tile_pool(name="pdw", bufs=2, space="PSUM") as pdw,    # [128,512] f32
            tc.tile_pool(name="ptp", bufs=2, space="PSUM") as ptp,    # [128,1024] bf16
            tc.tile_pool(name="psm", bufs=1, space="PSUM") as psm,    # [128,128] f32
        ):
            # ---- load x & weights (critical path first) ----
            xb_s = xpool.tile([128, 2, N], bf16)
            wqkv_s = wpool.tile([128, 2, 3 * C], bf16)
            for kb in range(2):
                nc.sync.dma_start(out=wqkv_s[:, kb, :], in_=wqkv_d[:, kb, :])
                nc.sync.dma_start(out=xb_s[:, kb, :], in_=xb_d[:, kb, :])
            wdiag_s = wpool.tile([128, 8, NB_QKV, 128], bf16)
            nc.sync.dma_start(out=wdiag_s, in_=wdiag_d[:, :, :, :])
            wdw_s = wpool.tile([128, 12, 9], f32)
            nc.sync.dma_start(out=wdw_s, in_=wdw_d[:, :, :])
            wproj_s = wpool.tile([128, 2, C], bf16)
            nc.sync.dma_start(out=wproj_s, in_=wproj_d[:, :, :])
            wm1_s = wpool.tile([128, 2, HID], bf16)
            nc.sync.dma_start(out=wm1_s, in_=wm1_d[:, :, :])
            wm2_s = wpool.tile([128, 3, C], bf16)
            nc.sync.dma_start(out=wm2_s, in_=wm2_d[:, :, :])
            b1_s = wpool.tile([128, 3], f32)
            nc.sync.dma_start(out=b1_s, in_=b1_d[:, :])
            b2_s = wpool.tile([128, 2], f32)
            nc.sync.dma_start(out=b2_s, in_=b2_d[:, :])
            tv_s = wpool.tile([128, 2], f32)
            nc.sync.dma_start(out=tv_s, in_=tv_d[:, :])

            ident = wpool.tile([128, 128], bf16)
            make_identity(nc, ident)

            dw_tiles = [None] * NB_QKV
            qT_s = qt_p.tile([128, 32, C], bf16, tag="qT")
            kT_s = qt_p.tile([128, 32, C], bf16, tag="kT")
            attn_s = [dwqk_p.tile([128, N], bf16, tag="dwqk", name=f"attn{g}")
                      for g in range(2)]
            nrm2 = small_p.tile([128, 4], f32, tag="nrm2")
            rnrm = small_p.tile([128, 4], f32, tag="rnrm")
            snrm = small_p.tile([128, 4], f32, tag="snrm")
            qsc = small_p.tile([128, 2], f32, tag="qsc")
            rs_v = [None, None]
            At_v = [None, None]

            def wap(ob, dy, dx, neg=False):
                t = (dy + 1) * 3 + (dx + 1)
                o = 6 + ob if neg else ob
                return wdw_s[:, o, t:t + 1]

            def do_block(ob):
                # qkv = W_qkv @ x -> PSUM [128,512] -> bf16 SBUF (ACT drain)
                qkvz = qkvp.tile([128, N + 2 * PAD], bf16, tag="qkv",
                                 name=f"qkv{ob}")
                nc.vector.memset(qkvz[:, 0:PAD], 0.0)
                nc.vector.memset(qkvz[:, PAD + N:], 0.0)
                qkv_t = qkvz[:, PAD:PAD + N]
                for t in range(8):
                    ps = pq.tile([128, TS], f32, tag="pq", name="ps")
                    for kb in range(2):
                        nc.tensor.matmul(
                            ps,
                            lhsT=wqkv_s[:, kb, ob * 128:(ob + 1) * 128],
                            rhs=xb_s[:, kb, t * TS:(t + 1) * TS],
                            start=(kb == 0), stop=(kb == 1),
                        )
                    nc.scalar.copy(out=qkv_t[:, t * TS:(t + 1) * TS], in_=ps)

                dw_t = (dwqk_p if ob < 4 else dwv_p).tile(
                    [128, N], bf16, tag=("dwqk" if ob < 4 else "dwv"),
                    name=f"dw{ob}")
                dw_tiles[ob] = dw_t
                dw3 = dw_t.rearrange("p (y x) -> p y x", y=H)
                qk3 = qkv_t.rearrange("p (y x) -> p y x", y=H)

                taps = PE_TAPS + (V_EXTRA if ob >= 4 else [])
                wm10 = wap(ob, -1, 0)
                for half in range(2):
                    for t8 in range(half * 4, half * 4 + 4):
                        pd = pdw.tile([128, TS], f32, tag="pdw", name="pd")
                        c0 = t8 * TS
                        for j, (dy, dx) in enumerate(taps):
                            s = dy * 64 + dx
                            nc.tensor.matmul(
                                pd,
                                lhsT=wdiag_s[:, j, ob, :],
                                rhs=qkvz[:, PAD + c0 + s:PAD + c0 + s + TS],
                                start=(j == 0), stop=(j == len(taps) - 1),
                            )
                        if t8 == 0:
                            # row 0 has no (-1,0) tap: plain drain on ACT
                            nc.scalar.copy(out=dw_t[:, 0:64], in_=pd[:, 0:64])
                            nc.vector.scalar_tensor_tensor(
                                out=dw_t[:, 64:TS], in0=qkv_t[:, 0:TS - 64],
                                scalar=wm10, in1=pd[:, 64:TS],
                                op0=OP.mult, op1=OP.add)
                        else:
                            nc.vector.scalar_tensor_tensor(
                                out=dw_t[:, c0:c0 + TS],
                                in0=qkv_t[:, c0 - 64:c0 + TS - 64],
                                scalar=wm10, in1=pd,
                                op0=OP.mult, op1=OP.add)
                    h0 = half * 2048
                    h1 = h0 + 2048
                    if ob < 4:
                        # taps (0,0) and (1,0) as TS(4x) + TT(2x) per half
                        sc = scr_p.tile([128, 2048], bf16, tag="ts",
                                        name=f"ts{ob}_{half}_0")
                        nc.vector.tensor_scalar_mul(
                            sc, qkv_t[:, h0:h1], wap(ob, 0, 0))
                        nc.vector.tensor_tensor(
                            out=dw_t[:, h0:h1], in0=dw_t[:, h0:h1], in1=sc,
                            op=OP.add)
                        e1 = min(h1, N - 64)
                        sc2 = scr_p.tile([128, 2048], bf16, tag="ts",
                                         name=f"ts{ob}_{half}_1")
                        nc.vector.tensor_scalar_mul(
                            sc2[:, 0:e1 - h0], qkv_t[:, h0 + 64:e1 + 64],
                            wap(ob, 1, 0))
                        nc.vector.tensor_tensor(
                            out=dw_t[:, h0:e1], in0=dw_t[:, h0:e1],
                            in1=sc2[:, 0:e1 - h0], op=OP.add)
                    # x-wrap column fixes for this half (negated weights)
                    yh0, yh1 = half * 32, half * 32 + 32
                    for (dy, dx), y0, y1, ox, idy, ix in FIXES:
                        ya, yb = max(y0, yh0), min(y1, yh1)
                        if ya < yb:
                            nc.vector.scalar_tensor_tensor(
                                out=dw3[:, ya:yb, ox:ox + 1],
                                in0=qk3[:, ya + idy:yb + idy, ix:ix + 1],
                                scalar=wap(ob, dy, dx, neg=True),
                                in1=dw3[:, ya:yb, ox:ox + 1],
                                op0=OP.mult, op1=OP.add)

                if ob < 4:
                    # sum of squares (ACT Square + accumulator)
                    sq = scr_p.tile([128, N], bf16, tag="sq")
                    nc.scalar.activation(
                        out=sq, in_=dw_t, func=AF.Square,
                        accum_out=nrm2[:, ob:ob + 1])
                if ob < 2:
                    do_transpose(ob)

            def do_transpose(ob):
                dw_t = dw_tiles[ob]
                dst = qT_s if ob < 2 else kT_s
                cof = (ob % 2) * 128
                for g in range(4):
                    tp_t = ptp.tile([128, 1024], bf16, tag="tp")
                    for i in range(8):
                        nb = g * 8 + i
                        nc.tensor.transpose(
                            tp_t[:, i * 128:(i + 1) * 128],
                            dw_t[:, nb * 128:(nb + 1) * 128], ident)
                    nc.vector.tensor_copy(
                        out=dst[:, g * 8:g * 8 + 8, cof:cof + 128],
                        in_=tp_t.rearrange("p (a b) -> p a b", a=8))

            def do_norms():
                # batched sqrt/recip for all four q/k norms; k-row scales
                nc.scalar.activation(out=snrm, in_=nrm2, func=AF.Sqrt)
                nc.vector.reciprocal(rnrm, snrm)
                nc.vector.tensor_mul(qsc, rnrm[:, 0:2], tv_s)
                for ob in (2, 3):
                    nc.vector.tensor_scalar_mul(
                        dw_tiles[ob], dw_tiles[ob], rnrm[:, ob:ob + 1])
                    do_transpose(ob)

            def do_gram(g):
                pg = psm.tile([128, 128], f32, tag="pg")
                co = g * 128
                for nb in range(32):
                    nc.tensor.matmul(
                        pg,
                        lhsT=qT_s[:, nb, co:co + 128],
                        rhs=kT_s[:, nb, co:co + 128],
                        start=(nb == 0), stop=(nb == 31),
                    )
                A_t = a_p.tile([128, 128], bf16, tag="A")
                nc.vector.memset(A_t, 0.0)
                sm = small_p.tile([128, 1], f32, tag="sm")
                for h in range(4):
                    r0, r1 = h * 32, h * 32 + 32
                    nc.scalar.activation(
                        out=A_t[r0:r1, r0:r1], in_=pg[r0:r1, r0:r1],
                        func=AF.Exp, scale=qsc[r0:r1, g:g + 1],
                        accum_out=sm[r0:r1, :])
                rs = small_p.tile([128, 1], f32, tag="rs", name=f"rs{g}")
                nc.vector.reciprocal(rs, sm)
                rs_v[g] = rs
                pa_t = ptp.tile([128, 1024], bf16, tag="tp")
                pa = pa_t[:, 0:128]
                nc.tensor.transpose(pa, A_t, ident)
                At = a_p.tile([128, 128], bf16, tag="At", name=f"At{g}")
                nc.vector.tensor_copy(out=At, in_=pa)
                At_v[g] = At

            def do_av_tile(g, t):
                pv = pq.tile([128, TS], f32, tag="pq", name="pv")
                nc.tensor.matmul(
                    pv, lhsT=At_v[g],
                    rhs=dw_tiles[4 + g][:, t * TS:(t + 1) * TS],
                    start=True, stop=True)
                nc.scalar.mul(attn_s[g][:, t * TS:(t + 1) * TS],
                              pv, rs_v[g])

            do_block(0)
            do_block(1)
            do_block(2)
            do_block(3)
            do_norms()
            do_gram(0)
            do_gram(1)
            do_block(4)
            for t in range(8):
                do_av_tile(0, t)
            do_block(5)

            # ---- streamed tail (av1 fused): per 512-tile ----
            x1b = [dwv_p.tile([128, N], bf16, tag="dwv", name=f"x1b{i}")
                   for i in range(2)]
            ys = [qkvp.tile([128, N], bf16, tag="qkv", name=f"ys{i}")
                  for i in range(3)]
            for t in range(8):
                sl = slice(t * TS, (t + 1) * TS)
                do_av_tile(1, t)
                for ob in range(2):
                    pp = pq.tile([128, TS], f32, tag="pq", name="pp")
                    for kb in range(2):
                        nc.tensor.matmul(
                            pp,
                            lhsT=wproj_s[:, kb, ob * 128:(ob + 1) * 128],
                            rhs=attn_s[kb][:, sl],
                            start=(kb == 0), stop=(kb == 1))
                    nc.vector.tensor_tensor(
                        out=x1b[ob][:, sl], in0=pp, in1=xb_s[:, ob, sl],
                        op=OP.add)
                for mb in range(3):
                    rows = 128 if mb < 2 else HID - 256
                    pm = pq.tile([128, TS], f32, tag="pq", name="pm")
                    for kb in range(2):
                        nc.tensor.matmul(
                            pm[:rows, :],
                            lhsT=wm1_s[:, kb, mb * 128:mb * 128 + rows],
                            rhs=x1b[kb][:, sl],
                            start=(kb == 0), stop=(kb == 1))
                    nc.scalar.activation(
                        out=ys[mb][:rows, sl],
                        in_=pm[:rows, :], func=AF.Gelu_apprx_tanh,
                        bias=b1_s[:rows, mb:mb + 1])
                for ob in range(2):
                    pm2 = pq.tile([128, TS], f32, tag="pq", name="pm2")
                    for kb in range(3):
                        rows = 128 if kb < 2 else HID - 256
                        nc.tensor.matmul(
                            pm2,
                            lhsT=wm2_s[:rows, kb, ob * 128:(ob + 1) * 128],
                            rhs=ys[kb][:rows, sl],
                            start=(kb == 0), stop=(kb == 2))
                    nc.vector.scalar_tensor_tensor(
                        out=xb_s[:, ob, sl], in0=pm2,
                        scalar=b2_s[:, ob:ob + 1], in1=x1b[ob][:, sl],
                        op0=OP.add, op1=OP.add)
                    nc.sync.dma_start(out=out_d[:, ob, sl], in_=xb_s[:, ob, sl])

    return nc


def _prep_shared(w_qkv, w_dw, temperature, w_proj, w_mlp1, b_mlp1, w_mlp2, b_mlp2):
    f32 = np.float32
    shared = {}
    shared["wqkvT"] = np.ascontiguousarray(
        w_qkv.T.reshape(2, 128, 3 * C).transpose(1, 0, 2)).astype(BF16)
    wd = np.zeros((128, 8, NB_QKV, 128), BF16)
    for ti, (dy, dx) in enumerate(PE_TAPS + V_EXTRA):
        for cb in range(NB_QKV):
            w = w_dw[cb * 128:(cb + 1) * 128, 0, dy + 1, dx + 1].astype(f32)
            wd[:, ti, cb, :] = np.diag(w).astype(BF16)
    shared["wdiag"] = wd
    wt = np.zeros((128, 12, 9), f32)
    for cb in range(NB_QKV):
        for t in range(9):
            wt[:, cb, t] = w_dw[cb * 128:(cb + 1) * 128, 0, t // 3, t % 3]
    wt[:, 6:, :] = -wt[:, :6, :]
    shared["wdw"] = wt
    shared["wprojT"] = np.ascontiguousarray(
        w_proj.T.reshape(2, 128, C).transpose(1, 0, 2)).astype(BF16)
    shared["wm1T"] = np.ascontiguousarray(
        w_mlp1.T.reshape(2, 128, HID).transpose(1, 0, 2)).astype(BF16)
    w2 = np.zeros((384, C), f32)
    w2[:HID] = w_mlp2.T
    shared["wm2T"] = np.ascontiguousarray(
        w2.reshape(3, 128, C).transpose(1, 0, 2)).astype(BF16)
    b1 = np.zeros((384,), f32)
    b1[:HID] = b_mlp1
    shared["b1"] = np.ascontiguousarray(b1.reshape(3, 128).T)
    shared["b2"] = np.ascontiguousarray(b_mlp2.astype(f32).reshape(2, 128).T)
    t = temperature.reshape(NH).astype(f32)
    tv = np.zeros((128, 2), f32)
    for g in range(2):
        tv[:, g] = np.repeat(t[g * 4:(g + 1) * 4], 32)
    shared["tempvec"] = tv
    return shared


def kernel(x, w_qkv, w_dw, temperature, w_proj, w_mlp1, b_mlp1, w_mlp2, b_mlp2,
           _trace=False):
    from concourse.bass_utils import run_bass_kernel_spmd

    if "nc" not in _CACHE:
        nc = _build_bass()
        nc.finalize()
        _CACHE["nc"] = nc
    nc = _CACHE["nc"]

    x = np.asarray(x, np.float32)
    B = x.shape[0]
    shared = _prep_shared(
        np.asarray(w_qkv, np.float32), np.asarray(w_dw, np.float32),
        np.asarray(temperature, np.float32), np.asarray(w_proj, np.float32),
        np.asarray(w_mlp1, np.float32), np.asarray(b_mlp1, np.float32),
        np.asarray(w_mlp2, np.float32), np.asarray(b_mlp2, np.float32))

    in_maps = []
    for i in range(B):
        m = dict(shared)
        xi = np.ascontiguousarray(x[i].reshape(2, 128, N).transpose(1, 0, 2))
        m["xb"] = xi.astype(BF16)
        in_maps.append(m)

    res = run_bass_kernel_spmd(nc, in_maps, core_ids=list(range(B)),
                               trace=_trace)
    outs = np.stack([
        np.asarray(r["out"]).astype(np.float32).transpose(1, 0, 2)
        .reshape(C, H, W) for r in res.results
    ])
    if _trace:
        _CACHE["last_exec_ns"] = res.exec_time_ns
        _CACHE["last_profile"] = res.profile_json
    return outs


# revision 33
# speedup vs baseline: 1.5095x; 1.0281x over previous
"""Trainium2 Bass kernel for nn_CustomABlock (MDTA transformer block).

Per-core layout: one batch image [C=256, N=4096(=64x64)] per NeuronCore,
data-parallel over B=8 across 8 cores, all params replicated.

dwconv 3x3 tap split (s = 64*dy + dx, flat shift over zero-padded qkv):
  q/k blocks: 6 PE diag-matmul taps (corners + (0,+-1)); DVE: (-1,0) folded
    into the PSUM-drain STT, (0,0) and (1,0) as TS(4x)+TT(2x) pairs.
  v blocks: 8 PE taps (adds (0,0),(1,0)); DVE only drains + fixes —
    frees DVE in the attention/tail transition.
  6 x-wrap column fixes per block (negated weights), all DVE work split
  per half-block so downstream av/tail unlock early.
Residuals in bf16 (x loaded bf16 only); softmax without the row-max pass
(|logit| <= t by Cauchy-Schwarz on normalized q,k); temperature/|q| folded
into the EXP scale; |k| scale applied once per k block; all four
sqrt/recips batched into one op each to minimize ACT table loads.
"""

import numpy as np
import ml_dtypes

BF16 = ml_dtypes.bfloat16

C = 256          # dim
N = 4096         # 64*64
H = W = 64
NH = 8           # heads
HID = 307        # mlp hidden
NB_QKV = 6       # qkv channel blocks of 128
TS = 512
PAD = 66         # zero pad elems each side of qkv (covers |s| <= 65, even)

# PE taps: flat full-range diag matmuls over the zero-padded qkv buffer.
PE_TAPS = [(0, -1), (0, 1), (-1, -1), (-1, 1), (1, -1), (1, 1)]
V_EXTRA = [(0, 0), (1, 0)]           # extra PE taps for v blocks
# x-wrap column fixes for PE taps: (tap, out_y0, out_y1, out_x, in_dy, in_x)
#   dw3[:, y0:y1, ox] -= w * qk3[:, y0+idy:y1+idy, ix]
FIXES = [
    ((0, -1), 1, 64, 0, -1, 63),
    ((-1, -1), 2, 64, 0, -2, 63),
    ((1, -1), 0, 64, 0, 0, 63),
    ((0, 1), 0, 63, 63, 1, 0),
    ((-1, 1), 0, 64, 63, 0, 0),
    ((1, 1), 0, 62, 63, 2, 0),
]

_CACHE = {}


def _build_bass():
    import os
    K_DMAADD = os.environ.get("KDMAADD", "0") == "1"
    import concourse.bass as bass
    from concourse import bacc
    from concourse import mybir
    from concourse.tile import TileContext
    from concourse.masks import make_identity

    dt = mybir.dt
    f32 = dt.float32
    bf16 = dt.bfloat16
    AF = mybir.ActivationFunctionType
    OP = mybir.AluOpType

    nc = bacc.Bacc("TRN2")

    # ---- DRAM I/O (per-core) ----
    xb_d = nc.dram_tensor("xb", [128, 2, N], bf16, kind="ExternalInput")
    wqkv_d = nc.dram_tensor("wqkvT", [128, 2, 3 * C], bf16, kind="ExternalInput")
    wdiag_d = nc.dram_tensor("wdiag", [128, 8, NB_QKV, 128], bf16,
                             kind="ExternalInput")
    # wdw[:, ob, t]: 9 taps fp32 per block; wdw[:, 6+ob, t]: negated
    wdw_d = nc.dram_tensor("wdw", [128, 12, 9], f32, kind="ExternalInput")
    wproj_d = nc.dram_tensor("wprojT", [128, 2, C], bf16, kind="ExternalInput")
    wm1_d = nc.dram_tensor("wm1T", [128, 2, HID], bf16, kind="ExternalInput")
    wm2_d = nc.dram_tensor("wm2T", [128, 3, C], bf16, kind="ExternalInput")
    b1_d = nc.dram_tensor("b1", [128, 3], f32, kind="ExternalInput")
    b2_d = nc.dram_tensor("b2", [128, 2], f32, kind="ExternalInput")
    tv_d = nc.dram_tensor("tempvec", [128, 2], f32, kind="ExternalInput")
    out_d = nc.dram_tensor("out", [128, 2, N], bf16, kind="ExternalOutput")

    with TileContext(nc) as tc:
        with (
            tc.tile_pool(name="wpool", bufs=1) as wpool,
            tc.tile_pool(name="xpool", bufs=1) as xpool,
            tc.tile_pool(name="qkvp", bufs=3) as qkvp,      # qkv blocks / ys reuse
            tc.tile_pool(name="dwqk", bufs=4) as dwqk_p,    # dw q/k blocks / attn_s
            tc.tile_pool(name="dwv", bufs=4) as dwv_p,      # dw v blocks / x1b reuse
            tc.tile_pool(name="qt", bufs=1) as qt_p,
            tc.tile_pool(name="scr", bufs=2) as scr_p,
            tc.tile_pool(name="small", bufs=12) as small_p,
            tc.tile_pool(name="apool", bufs=2) as a_p,
            tc.tile_pool(name="pq", bufs=3, space="PSUM") as pq,      # [128,512] f32
            tc.tile_pool(name="pdw", bufs=2, space="PSUM") as pdw,    # [128,512] f32
            tc.tile_pool(name="ptp", bufs=2, space="PSUM") as ptp,    # [128,1024] bf16
            tc.tile_pool(name="psm", bufs=1, space="PSUM") as psm,    # [128,128] f32
        ):
            # ---- load x & weights (critical path first) ----
            xb_s = xpool.tile([128, 2, N], bf16)
            wqkv_s = wpool.tile([128, 2, 3 * C], bf16)
            for kb in range(2):
                nc.sync.dma_start(out=wqkv_s[:, kb, :], in_=wqkv_d[:, kb, :])
                nc.sync.dma_start(out=xb_s[:, kb, :], in_=xb_d[:, kb, :])
            wdiag_s = wpool.tile([128, 8, NB_QKV, 128], bf16)
            nc.sync.dma_start(out=wdiag_s, in_=wdiag_d[:, :, :, :])
            wdw_s = wpool.tile([128, 12, 9], f32)
            nc.sync.dma_start(out=wdw_s, in_=wdw_d[:, :, :])
            wproj_s = wpool.tile([128, 2, C], bf16)
            nc.sync.dma_start(out=wproj_s, in_=wproj_d[:, :, :])
            wm1_s = wpool.tile([128, 2, HID], bf16)
            nc.sync.dma_start(out=wm1_s, in_=wm1_d[:, :, :])
            wm2_s = wpool.tile([128, 3, C], bf16)
            nc.sync.dma_start(out=wm2_s, in_=wm2_d[:, :, :])
            b1_s = wpool.tile([128, 3], f32)
            nc.sync.dma_start(out=b1_s, in_=b1_d[:, :])
            b2_s = wpool.tile([128, 2], f32)
            nc.sync.dma_start(out=b2_s, in_=b2_d[:, :])
            tv_s = wpool.tile([128, 2], f32)
            nc.sync.dma_start(out=tv_s, in_=tv_d[:, :])

            ident = wpool.tile([128, 128], bf16)
            make_identity(nc, ident)

            dw_tiles = [None] * NB_QKV
            qT_s = qt_p.tile([128, 32, C], bf16, tag="qT")
            kT_s = qt_p.tile([128, 32, C], bf16, tag="kT")
            attn_s = [dwqk_p.tile([128, N], bf16, tag="dwqk", name=f"attn{g}")
                      for g in range(2)]
            nrm2 = small_p.tile([128, 4], f32, tag="nrm2")
            rnrm = small_p.tile([128, 4], f32, tag="rnrm")
            snrm = small_p.tile([128, 4], f32, tag="snrm")
            qsc = small_p.tile([128, 2], f32, tag="qsc")
            rs_v = [None, None]
            At_v = [None, None]

            def wap(ob, dy, dx, neg=False):
                t = (dy + 1) * 3 + (dx + 1)
                o = 6 + ob if neg else ob
                return wdw_s[:, o, t:t + 1]

            def do_block(ob):
                # qkv = W_qkv @ x -> PSUM [128,512] -> bf16 SBUF (ACT drain)
                qkvz = qkvp.tile([128, N + 2 * PAD], bf16, tag="qkv",
                                 name=f"qkv{ob}")
                nc.vector.memset(qkvz[:, 0:PAD], 0.0)
                nc.vector.memset(qkvz[:, PAD + N:], 0.0)
                qkv_t = qkvz[:, PAD:PAD + N]
                for t in range(8):
                    ps = pq.tile([128, TS], f32, tag="pq", name="ps")
                    for kb in range(2):
                        nc.tensor.matmul(
                            ps,
                            lhsT=wqkv_s[:, kb, ob * 128:(ob + 1) * 128],
                            rhs=xb_s[:, kb, t * TS:(t + 1) * TS],
                            start=(kb == 0), stop=(kb == 1),
                        )
                    nc.scalar.copy(out=qkv_t[:, t * TS:(t + 1) * TS], in_=ps)

                dw_t = (dwqk_p if ob < 4 else dwv_p).tile(
                    [128, N], bf16, tag=("dwqk" if ob < 4 else "dwv"),
                    name=f"dw{ob}")
                dw_tiles[ob] = dw_t
                dw3 = dw_t.rearrange("p (y x) -> p y x", y=H)
                qk3 = qkv_t.rearrange("p (y x) -> p y x", y=H)

                taps = PE_TAPS + (V_EXTRA if ob >= 4 else [])
                wm10 = wap(ob, -1, 0)
                for half in range(2):
                    for t8 in range(half * 4, half * 4 + 4):
                        pd = pdw.tile([128, TS], f32, tag="pdw", name="pd")
                        c0 = t8 * TS
                        for j, (dy, dx) in enumerate(taps):
                            s = dy * 64 + dx
                            nc.tensor.matmul(
                                pd,
                                lhsT=wdiag_s[:, j, ob, :],
                                rhs=qkvz[:, PAD + c0 + s:PAD + c0 + s + TS],
                                start=(j == 0), stop=(j == len(taps) - 1),
                            )
                        if t8 == 0:
                            # row 0 has no (-1,0) tap: plain drain on ACT
                            nc.scalar.copy(out=dw_t[:, 0:64], in_=pd[:, 0:64])
                            nc.vector.scalar_tensor_tensor(
                                out=dw_t[:, 64:TS], in0=qkv_t[:, 0:TS - 64],
                                scalar=wm10, in1=pd[:, 64:TS],
                                op0=OP.mult, op1=OP.add)
                        else:
                            nc.vector.scalar_tensor_tensor(
                                out=dw_t[:, c0:c0 + TS],
                                in0=qkv_t[:, c0 - 64:c0 + TS - 64],
                                scalar=wm10, in1=pd,
                                op0=OP.mult, op1=OP.add)
                    h0 = half * 2048
                    h1 = h0 + 2048
                    if ob < 4:
                        # taps (0,0) and (1,0) as TS(4x) + TT(2x) per half
                        sc = scr_p.tile([128, 2048], bf16, tag="ts",
                                        name=f"ts{ob}_{half}_0")
                        nc.vector.tensor_scalar_mul(
                            sc, qkv_t[:, h0:h1], wap(ob, 0, 0))
                        e1 = min(h1, N - 64)
                        sc2 = scr_p.tile([128, 2048], bf16, tag="ts",
                                         name=f"ts{ob}_{half}_1")
                        nc.vector.tensor_scalar_mul(
                            sc2[:, 0:e1 - h0], qkv_t[:, h0 + 64:e1 + 64],
                            wap(ob, 1, 0))
                        if K_DMAADD:
                            nc.gpsimd.dma_start(
                                out=dw_t[:, h0:h1], in_=sc, accum_op=OP.add)
                            nc.gpsimd.dma_start(
                                out=dw_t[:, h0:e1], in_=sc2[:, 0:e1 - h0],
                                accum_op=OP.add)
                        else:
                            nc.vector.tensor_tensor(
                                out=dw_t[:, h0:h1], in0=dw_t[:, h0:h1],
                                in1=sc, op=OP.add)
                            nc.vector.tensor_tensor(
                                out=dw_t[:, h0:e1], in0=dw_t[:, h0:e1],
                                in1=sc2[:, 0:e1 - h0], op=OP.add)
                    # x-wrap column fixes for this half (negated weights)
                    yh0, yh1 = half * 32, half * 32 + 32
                    for (dy, dx), y0, y1, ox, idy, ix in FIXES:
                        ya, yb = max(y0, yh0), min(y1, yh1)
                        if ya < yb:
                            nc.vector.scalar_tensor_tensor(
                                out=dw3[:, ya:yb, ox:ox + 1],
                                in0=qk3[:, ya + idy:yb + idy, ix:ix + 1],
                                scalar=wap(ob, dy, dx, neg=True),
                                in1=dw3[:, ya:yb, ox:ox + 1],
                                op0=OP.mult, op1=OP.add)

                if ob < 4:
                    # sum of squares (ACT Square + accumulator)
                    sq = scr_p.tile([128, N], bf16, tag="sq")
                    nc.scalar.activation(
                        out=sq, in_=dw_t, func=AF.Square,
                        accum_out=nrm2[:, ob:ob + 1])
                if ob < 2:
                    do_transpose(ob)

            def do_transpose(ob):
                dw_t = dw_tiles[ob]
                dst = qT_s if ob < 2 else kT_s
                cof = (ob % 2) * 128
                for g in range(4):
                    tp_t = ptp.tile([128, 1024], bf16, tag="tp")
                    for i in range(8):
                        nb = g * 8 + i
                        nc.tensor.transpose(
                            tp_t[:, i * 128:(i + 1) * 128],
                            dw_t[:, nb * 128:(nb + 1) * 128], ident)
                    nc.vector.tensor_copy(
                        out=dst[:, g * 8:g * 8 + 8, cof:cof + 128],
                        in_=tp_t.rearrange("p (a b) -> p a b", a=8))

            def do_norms():
                # batched sqrt/recip for all four q/k norms; k-row scales
                nc.scalar.activation(out=snrm, in_=nrm2, func=AF.Sqrt)
                nc.vector.reciprocal(rnrm, snrm)
                nc.vector.tensor_mul(qsc, rnrm[:, 0:2], tv_s)
                for ob in (2, 3):
                    nc.vector.tensor_scalar_mul(
                        dw_tiles[ob], dw_tiles[ob], rnrm[:, ob:ob + 1])
                    do_transpose(ob)

            def do_gram(g):
                pg = psm.tile([128, 128], f32, tag="pg")
                co = g * 128
                for nb in range(32):
                    nc.tensor.matmul(
                        pg,
                        lhsT=qT_s[:, nb, co:co + 128],
                        rhs=kT_s[:, nb, co:co + 128],
                        start=(nb == 0), stop=(nb == 31),
                    )
                A_t = a_p.tile([128, 128], bf16, tag="A")
                nc.vector.memset(A_t, 0.0)
                sm = small_p.tile([128, 1], f32, tag="sm")
                for h in range(4):
                    r0, r1 = h * 32, h * 32 + 32
                    nc.scalar.activation(
                        out=A_t[r0:r1, r0:r1], in_=pg[r0:r1, r0:r1],
                        func=AF.Exp, scale=qsc[r0:r1, g:g + 1],
                        accum_out=sm[r0:r1, :])
                rs = small_p.tile([128, 1], f32, tag="rs", name=f"rs{g}")
                nc.vector.reciprocal(rs, sm)
                rs_v[g] = rs
                pa_t = ptp.tile([128, 1024], bf16, tag="tp")
                pa = pa_t[:, 0:128]
                nc.tensor.transpose(pa, A_t, ident)
                At = a_p.tile([128, 128], bf16, tag="At", name=f"At{g}")
                nc.vector.tensor_copy(out=At, in_=pa)
                At_v[g] = At

            def do_av_tile(g, t):
                pv = pq.tile([128, TS], f32, tag="pq", name="pv")
                nc.tensor.matmul(
                    pv, lhsT=At_v[g],
                    rhs=dw_tiles[4 + g][:, t * TS:(t + 1) * TS],
                    start=True, stop=True)
                nc.scalar.mul(attn_s[g][:, t * TS:(t + 1) * TS],
                              pv, rs_v[g])

            do_block(0)
            do_block(1)
            do_block(2)
            do_block(3)
            do_block(4)
            do_norms()
            do_gram(0)
            do_gram(1)
            do_block(5)

            # ---- streamed tail (av0+av1 fused): per 512-tile ----
            x1b = [dwv_p.tile([128, N], bf16, tag="dwv", name=f"x1b{i}")
                   for i in range(2)]
            ys = [qkvp.tile([128, N], bf16, tag="qkv", name=f"ys{i}")
                  for i in range(3)]
            for t in range(8):
                sl = slice(t * TS, (t + 1) * TS)
                do_av_tile(0, t)
                do_av_tile(1, t)
                for ob in range(2):
                    pp = pq.tile([128, TS], f32, tag="pq", name="pp")
                    for kb in range(2):
                        nc.tensor.matmul(
                            pp,
                            lhsT=wproj_s[:, kb, ob * 128:(ob + 1) * 128],
                            rhs=attn_s[kb][:, sl],
                            start=(kb == 0), stop=(kb == 1))
                    nc.vector.tensor_tensor(
                        out=x1b[ob][:, sl], in0=pp, in1=xb_s[:, ob, sl],
                        op=OP.add)
                for mb in range(3):
                    rows = 128 if mb < 2 else HID - 256
                    pm = pq.tile([128, TS], f32, tag="pq", name="pm")
                    for kb in range(2):
                        nc.tensor.matmul(
                            pm[:rows, :],
                            lhsT=wm1_s[:, kb, mb * 128:mb * 128 + rows],
                            rhs=x1b[kb][:, sl],
                            start=(kb == 0), stop=(kb == 1))
                    nc.scalar.activation(
                        out=ys[mb][:rows, sl],
                        in_=pm[:rows, :], func=AF.Gelu_apprx_tanh,
                        bias=b1_s[:rows, mb:mb + 1])
                for ob in range(2):
                    pm2 = pq.tile([128, TS], f32, tag="pq", name="pm2")
                    for kb in range(3):
                        rows = 128 if kb < 2 else HID - 256
                        nc.tensor.matmul(
                            pm2,
                            lhsT=wm2_s[:rows, kb, ob * 128:(ob + 1) * 128],
                            rhs=ys[kb][:rows, sl],
                            start=(kb == 0), stop=(kb == 2))
                    nc.vector.scalar_tensor_tensor(
                        out=xb_s[:, ob, sl], in0=pm2,
                        scalar=b2_s[:, ob:ob + 1], in1=x1b[ob][:, sl],
                        op0=OP.add, op1=OP.add)
                    nc.sync.dma_start(out=out_d[:, ob, sl], in_=xb_s[:, ob, sl])

    return nc


def _prep_shared(w_qkv, w_dw, temperature, w_proj, w_mlp1, b_mlp1, w_mlp2, b_mlp2):
    f32 = np.float32
    shared = {}
    shared["wqkvT"] = np.ascontiguousarray(
        w_qkv.T.reshape(2, 128, 3 * C).transpose(1, 0, 2)).astype(BF16)
    wd = np.zeros((128, 8, NB_QKV, 128), BF16)
    for ti, (dy, dx) in enumerate(PE_TAPS + V_EXTRA):
        for cb in range(NB_QKV):
            w = w_dw[cb * 128:(cb + 1) * 128, 0, dy + 1, dx + 1].astype(f32)
            wd[:, ti, cb, :] = np.diag(w).astype(BF16)
    shared["wdiag"] = wd
    wt = np.zeros((128, 12, 9), f32)
    for cb in range(NB_QKV):
        for t in range(9):
            wt[:, cb, t] = w_dw[cb * 128:(cb + 1) * 128, 0, t // 3, t % 3]
    wt[:, 6:, :] = -wt[:, :6, :]
    shared["wdw"] = wt
    shared["wprojT"] = np.ascontiguousarray(
        w_proj.T.reshape(2, 128, C).transpose(1, 0, 2)).astype(BF16)
    shared["wm1T"] = np.ascontiguousarray(
        w_mlp1.T.reshape(2, 128, HID).transpose(1, 0, 2)).astype(BF16)
    w2 = np.zeros((384, C), f32)
    w2[:HID] = w_mlp2.T
    shared["wm2T"] = np.ascontiguousarray(
        w2.reshape(3, 128, C).transpose(1, 0, 2)).astype(BF16)
    b1 = np.zeros((384,), f32)
    b1[:HID] = b_mlp1
    shared["b1"] = np.ascontiguousarray(b1.reshape(3, 128).T)
    shared["b2"] = np.ascontiguousarray(b_mlp2.astype(f32).reshape(2, 128).T)
    t = temperature.reshape(NH).astype(f32)
    tv = np.zeros((128, 2), f32)
    for g in range(2):
        tv[:, g] = np.repeat(t[g * 4:(g + 1) * 4], 32)
    shared["tempvec"] = tv
    return shared


def kernel(x, w_qkv, w_dw, temperature, w_proj, w_mlp1, b_mlp1, w_mlp2, b_mlp2,
           _trace=False):
    from concourse.bass_utils import run_bass_kernel_spmd

    if "nc" not in _CACHE:
        nc = _build_bass()
        nc.finalize()
        _CACHE["nc"] = nc
    nc = _CACHE["nc"]

    x = np.asarray(x, np.float32)
    B = x.shape[0]
    shared = _prep_shared(
        np.asarray(w_qkv, np.float32), np.asarray(w_dw, np.float32),
        np.asarray(temperature, np.float32), np.asarray(w_proj, np.float32),
        np.asarray(w_mlp1, np.float32), np.asarray(b_mlp1, np.float32),
        np.asarray(w_mlp2, np.float32), np.asarray(b_mlp2, np.float32))

    in_maps = []
    for i in range(B):
        m = dict(shared)
        xi = np.ascontiguousarray(x[i].reshape(2, 128, N).transpose(1, 0, 2))
        m["xb"] = xi.astype(BF16)
        in_maps.append(m)

    res = run_bass_kernel_spmd(nc, in_maps, core_ids=list(range(B)),
                               trace=_trace)
    outs = np.stack([
        np.asarray(r["out"]).astype(np.float32).transpose(1, 0, 2)
        .reshape(C, H, W) for r in res.results
    ])
    if _trace:
        _CACHE["last_exec_ns"] = res.exec_time_ns
        _CACHE["last_profile"] = res.profile_json
    return outs


# revision 35
# speedup vs baseline: 1.6440x; 1.0891x over previous
"""Trainium2 Bass kernel for nn_CustomABlock (MDTA transformer block).

Per-core layout: one batch image [C=256, N=4096(=64x64)] per NeuronCore,
data-parallel over B=8 across 8 cores, all params replicated.

dwconv 3x3 tap split (s = 64*dy + dx, flat shift over zero-padded qkv):
  q/k blocks: 6 PE diag-matmul taps (corners + (0,+-1)); DVE: (-1,0) folded
    into the PSUM-drain STT, (0,0) and (1,0) as TS(4x)+TT(2x) pairs.
  v blocks: 8 PE taps (adds (0,0),(1,0)); DVE only drains + fixes —
    frees DVE in the attention/tail transition.
  6 x-wrap column fixes per block (negated weights), all DVE work split
  per half-block so downstream av/tail unlock early.
Residuals in bf16 (x loaded bf16 only); softmax without the row-max pass
(|logit| <= t by Cauchy-Schwarz on normalized q,k); temperature/|q| folded
into the EXP scale; |k| scale applied once per k block; all four
sqrt/recips batched into one op each to minimize ACT table loads.
"""

import numpy as np
import ml_dtypes

BF16 = ml_dtypes.bfloat16

C = 256          # dim
N = 4096         # 64*64
H = W = 64
NH = 8           # heads
HID = 307        # mlp hidden
NB_QKV = 6       # qkv channel blocks of 128
TS = 512
PAD = 66         # zero pad elems each side of qkv (covers |s| <= 65, even)

# PE taps: flat full-range diag matmuls over the zero-padded qkv buffer.
PE_TAPS = [(0, -1), (0, 1), (-1, -1), (-1, 1), (1, -1), (1, 1)]
V_EXTRA = [(0, 0), (1, 0)]           # extra PE taps for v blocks
# x-wrap column fixes for PE taps: (tap, out_y0, out_y1, out_x, in_dy, in_x)
#   dw3[:, y0:y1, ox] -= w * qk3[:, y0+idy:y1+idy, ix]
FIXES = [
    ((0, -1), 1, 64, 0, -1, 63),
    ((-1, -1), 2, 64, 0, -2, 63),
    ((1, -1), 0, 64, 0, 0, 63),
    ((0, 1), 0, 63, 63, 1, 0),
    ((-1, 1), 0, 64, 63, 0, 0),
    ((1, 1), 0, 62, 63, 2, 0),
]

_CACHE = {}


def _build_bass():
    import os
    K_DMAADD = os.environ.get("KDMAADD", "0") == "1"
    import concourse.bass as bass
    from concourse import bacc
    from concourse import mybir
    from concourse.tile import TileContext
    from concourse.masks import make_identity

    dt = mybir.dt
    f32 = dt.float32
    bf16 = dt.bfloat16
    AF = mybir.ActivationFunctionType
    OP = mybir.AluOpType

    nc = bacc.Bacc("TRN2")

    # ---- DRAM I/O (per-core) ----
    xb_d = nc.dram_tensor("xb", [128, 2, N], bf16, kind="ExternalInput")
    wqkv_d = nc.dram_tensor("wqkvT", [128, 2, 3 * C], bf16, kind="ExternalInput")
    wdiag_d = nc.dram_tensor("wdiag", [128, 8, NB_QKV, 128], bf16,
                             kind="ExternalInput")
    # wdw[:, ob, t]: 9 taps fp32 per block; wdw[:, 6+ob, t]: negated
    wdw_d = nc.dram_tensor("wdw", [128, 12, 9], f32, kind="ExternalInput")
    wproj_d = nc.dram_tensor("wprojT", [128, 2, C], bf16, kind="ExternalInput")
    wm1_d = nc.dram_tensor("wm1T", [128, 2, HID], bf16, kind="ExternalInput")
    wm2_d = nc.dram_tensor("wm2T", [128, 3, C], bf16, kind="ExternalInput")
    b1_d = nc.dram_tensor("b1", [128, 3], f32, kind="ExternalInput")
    b2_d = nc.dram_tensor("b2", [128, 2], f32, kind="ExternalInput")
    tv_d = nc.dram_tensor("tempvec", [128, 2], f32, kind="ExternalInput")
    out_d = nc.dram_tensor("out", [128, 2, N], bf16, kind="ExternalOutput")

    with TileContext(nc) as tc:
        with (
            tc.tile_pool(name="wpool", bufs=1) as wpool,
            tc.tile_pool(name="xpool", bufs=1) as xpool,
            tc.tile_pool(name="qkvp", bufs=3) as qkvp,      # qkv blocks / ys reuse
            tc.tile_pool(name="dwqk", bufs=4) as dwqk_p,    # dw q/k blocks / attn_s
            tc.tile_pool(name="dwv", bufs=4) as dwv_p,      # dw v blocks / x1b reuse
            tc.tile_pool(name="qt", bufs=1) as qt_p,
            tc.tile_pool(name="scr", bufs=2) as scr_p,
            tc.tile_pool(name="small", bufs=12) as small_p,
            tc.tile_pool(name="apool", bufs=2) as a_p,
            tc.tile_pool(name="pq", bufs=3, space="PSUM") as pq,      # [128,512] f32
            tc.tile_pool(name="pdw", bufs=2, space="PSUM") as pdw,    # [128,512] f32
            tc.tile_pool(name="ptp", bufs=2, space="PSUM") as ptp,    # [128,1024] bf16
            tc.tile_pool(name="psm", bufs=1, space="PSUM") as psm,    # [128,128] f32
        ):
            # ---- load x & weights (critical path first) ----
            xb_s = xpool.tile([128, 2, N], bf16)
            wqkv_s = wpool.tile([128, 2, 3 * C], bf16)
            for kb in range(2):
                nc.sync.dma_start(out=wqkv_s[:, kb, :], in_=wqkv_d[:, kb, :])
            for h in range(2):
                for kb in range(2):
                    nc.sync.dma_start(
                        out=xb_s[:, kb, h * 2048:(h + 1) * 2048],
                        in_=xb_d[:, kb, h * 2048:(h + 1) * 2048])
            wdiag_s = wpool.tile([128, 8, NB_QKV, 128], bf16)
            nc.sync.dma_start(out=wdiag_s, in_=wdiag_d[:, :, :, :])
            wdw_s = wpool.tile([128, 12, 9], f32)
            nc.sync.dma_start(out=wdw_s, in_=wdw_d[:, :, :])
            wproj_s = wpool.tile([128, 2, C], bf16)
            nc.sync.dma_start(out=wproj_s, in_=wproj_d[:, :, :])
            wm1_s = wpool.tile([128, 2, HID], bf16)
            nc.sync.dma_start(out=wm1_s, in_=wm1_d[:, :, :])
            wm2_s = wpool.tile([128, 3, C], bf16)
            nc.sync.dma_start(out=wm2_s, in_=wm2_d[:, :, :])
            b1_s = wpool.tile([128, 3], f32)
            nc.sync.dma_start(out=b1_s, in_=b1_d[:, :])
            b2_s = wpool.tile([128, 2], f32)
            nc.sync.dma_start(out=b2_s, in_=b2_d[:, :])
            tv_s = wpool.tile([128, 2], f32)
            nc.sync.dma_start(out=tv_s, in_=tv_d[:, :])

            ident = wpool.tile([128, 128], bf16)
            make_identity(nc, ident)

            dw_tiles = [None] * NB_QKV
            qT_s = qt_p.tile([128, 32, C], bf16, tag="qT")
            kT_s = qt_p.tile([128, 32, C], bf16, tag="kT")
            attn_s = [dwqk_p.tile([128, N], bf16, tag="dwqk", name=f"attn{g}")
                      for g in range(2)]
            nrm2 = small_p.tile([128, 4], f32, tag="nrm2")
            rnrm = small_p.tile([128, 4], f32, tag="rnrm")
            snrm = small_p.tile([128, 4], f32, tag="snrm")
            qsc = small_p.tile([128, 2], f32, tag="qsc")
            rs_v = [None, None]
            At_v = [None, None]

            def wap(ob, dy, dx, neg=False):
                t = (dy + 1) * 3 + (dx + 1)
                o = 6 + ob if neg else ob
                return wdw_s[:, o, t:t + 1]

            def do_block(ob):
                # qkv = W_qkv @ x -> PSUM [128,512] -> bf16 SBUF (ACT drain)
                qkvz = qkvp.tile([128, N + 2 * PAD], bf16, tag="qkv",
                                 name=f"qkv{ob}")
                nc.vector.memset(qkvz[:, 0:PAD], 0.0)
                nc.vector.memset(qkvz[:, PAD + N:], 0.0)
                qkv_t = qkvz[:, PAD:PAD + N]
                for t in range(8):
                    ps = pq.tile([128, TS], f32, tag="pq", name="ps")
                    for kb in range(2):
                        nc.tensor.matmul(
                            ps,
                            lhsT=wqkv_s[:, kb, ob * 128:(ob + 1) * 128],
                            rhs=xb_s[:, kb, t * TS:(t + 1) * TS],
                            start=(kb == 0), stop=(kb == 1),
                        )
                    nc.scalar.copy(out=qkv_t[:, t * TS:(t + 1) * TS], in_=ps)

                dw_t = (dwqk_p if ob < 4 else dwv_p).tile(
                    [128, N], bf16, tag=("dwqk" if ob < 4 else "dwv"),
                    name=f"dw{ob}")
                dw_tiles[ob] = dw_t
                dw3 = dw_t.rearrange("p (y x) -> p y x", y=H)
                qk3 = qkv_t.rearrange("p (y x) -> p y x", y=H)

                taps = PE_TAPS + (V_EXTRA if ob >= 4 else [])
                wm10 = wap(ob, -1, 0)
                for half in range(2):
                    for t8 in range(half * 4, half * 4 + 4):
                        pd = pdw.tile([128, TS], f32, tag="pdw", name="pd")
                        c0 = t8 * TS
                        for j, (dy, dx) in enumerate(taps):
                            s = dy * 64 + dx
                            nc.tensor.matmul(
                                pd,
                                lhsT=wdiag_s[:, j, ob, :],
                                rhs=qkvz[:, PAD + c0 + s:PAD + c0 + s + TS],
                                start=(j == 0), stop=(j == len(taps) - 1),
                            )
                        if t8 == 0:
                            # row 0 has no (-1,0) tap: plain drain on ACT
                            nc.scalar.copy(out=dw_t[:, 0:64], in_=pd[:, 0:64])
                            nc.vector.scalar_tensor_tensor(
                                out=dw_t[:, 64:TS], in0=qkv_t[:, 0:TS - 64],
                                scalar=wm10, in1=pd[:, 64:TS],
                                op0=OP.mult, op1=OP.add)
                        else:
                            nc.vector.scalar_tensor_tensor(
                                out=dw_t[:, c0:c0 + TS],
                                in0=qkv_t[:, c0 - 64:c0 + TS - 64],
                                scalar=wm10, in1=pd,
                                op0=OP.mult, op1=OP.add)
                    h0 = half * 2048
                    h1 = h0 + 2048
                    if ob < 4:
                        # taps (0,0) and (1,0) as TS(4x) + TT(2x) per half
                        sc = scr_p.tile([128, 2048], bf16, tag="ts",
                                        name=f"ts{ob}_{half}_0")
                        nc.vector.tensor_scalar_mul(
                            sc, qkv_t[:, h0:h1], wap(ob, 0, 0))
                        e1 = min(h1, N - 64)
                        sc2 = scr_p.tile([128, 2048], bf16, tag="ts",
                                         name=f"ts{ob}_{half}_1")
                        nc.vector.tensor_scalar_mul(
                            sc2[:, 0:e1 - h0], qkv_t[:, h0 + 64:e1 + 64],
                            wap(ob, 1, 0))
                        if K_DMAADD:
                            nc.gpsimd.dma_start(
                                out=dw_t[:, h0:h1], in_=sc, accum_op=OP.add)
                            nc.gpsimd.dma_start(
                                out=dw_t[:, h0:e1], in_=sc2[:, 0:e1 - h0],
                                accum_op=OP.add)
                        else:
                            nc.vector.tensor_tensor(
                                out=dw_t[:, h0:h1], in0=dw_t[:, h0:h1],
                                in1=sc, op=OP.add)
                            nc.vector.tensor_tensor(
                                out=dw_t[:, h0:e1], in0=dw_t[:, h0:e1],
                                in1=sc2[:, 0:e1 - h0], op=OP.add)
                    # x-wrap column fixes for this half (negated weights)
                    yh0, yh1 = half * 32, half * 32 + 32
                    for (dy, dx), y0, y1, ox, idy, ix in FIXES:
                        ya, yb = max(y0, yh0), min(y1, yh1)
                        if ya < yb:
                            nc.vector.scalar_tensor_tensor(
                                out=dw3[:, ya:yb, ox:ox + 1],
                                in0=qk3[:, ya + idy:yb + idy, ix:ix + 1],
                                scalar=wap(ob, dy, dx, neg=True),
                                in1=dw3[:, ya:yb, ox:ox + 1],
                                op0=OP.mult, op1=OP.add)

                if ob < 4:
                    # sum of squares (ACT Square + accumulator)
                    sq = scr_p.tile([128, N], bf16, tag="sq")
                    nc.scalar.activation(
                        out=sq, in_=dw_t, func=AF.Square,
                        accum_out=nrm2[:, ob:ob + 1])
                if ob < 2:
                    do_transpose(ob)

            def do_transpose(ob):
                dw_t = dw_tiles[ob]
                dst = qT_s if ob < 2 else kT_s
                cof = (ob % 2) * 128
                for g in range(4):
                    tp_t = ptp.tile([128, 1024], bf16, tag="tp")
                    for i in range(8):
                        nb = g * 8 + i
                        nc.tensor.transpose(
                            tp_t[:, i * 128:(i + 1) * 128],
                            dw_t[:, nb * 128:(nb + 1) * 128], ident)
                    nc.vector.tensor_copy(
                        out=dst[:, g * 8:g * 8 + 8, cof:cof + 128],
                        in_=tp_t.rearrange("p (a b) -> p a b", a=8))

            def do_norms():
                # batched sqrt/recip for all four q/k norms; k-row scales
                nc.scalar.activation(out=snrm, in_=nrm2, func=AF.Sqrt)
                nc.vector.reciprocal(rnrm, snrm)
                nc.vector.tensor_mul(qsc, rnrm[:, 0:2], tv_s)
                for ob in (2, 3):
                    nc.vector.tensor_scalar_mul(
                        dw_tiles[ob], dw_tiles[ob], rnrm[:, ob:ob + 1])
                    do_transpose(ob)

            def do_gram(g):
                pg = psm.tile([128, 128], f32, tag="pg")
                co = g * 128
                for nb in range(32):
                    nc.tensor.matmul(
                        pg,
                        lhsT=qT_s[:, nb, co:co + 128],
                        rhs=kT_s[:, nb, co:co + 128],
                        start=(nb == 0), stop=(nb == 31),
                    )
                A_t = a_p.tile([128, 128], bf16, tag="A")
                nc.vector.memset(A_t, 0.0)
                sm = small_p.tile([128, 1], f32, tag="sm")
                for h in range(4):
                    r0, r1 = h * 32, h * 32 + 32
                    nc.scalar.activation(
                        out=A_t[r0:r1, r0:r1], in_=pg[r0:r1, r0:r1],
                        func=AF.Exp, scale=qsc[r0:r1, g:g + 1],
                        accum_out=sm[r0:r1, :])
                rs = small_p.tile([128, 1], f32, tag="rs", name=f"rs{g}")
                nc.vector.reciprocal(rs, sm)
                rs_v[g] = rs
                pa_t = ptp.tile([128, 1024], bf16, tag="tp")
                pa = pa_t[:, 0:128]
                nc.tensor.transpose(pa, A_t, ident)
                At = a_p.tile([128, 128], bf16, tag="At", name=f"At{g}")
                nc.vector.tensor_copy(out=At, in_=pa)
                At_v[g] = At

            def do_av_tile(g, t):
                pv = pq.tile([128, TS], f32, tag="pq", name="pv")
                nc.tensor.matmul(
                    pv, lhsT=At_v[g],
                    rhs=dw_tiles[4 + g][:, t * TS:(t + 1) * TS],
                    start=True, stop=True)
                nc.scalar.mul(attn_s[g][:, t * TS:(t + 1) * TS],
                              pv, rs_v[g])

            do_block(0)
            do_block(1)
            do_block(2)
            do_block(3)
            do_block(4)
            do_norms()
            do_gram(0)
            do_gram(1)
            do_block(5)

            # ---- software-pipelined tail: av -> proj -> mlp1 -> mlp2 ----
            x1b = [dwv_p.tile([128, N], bf16, tag="dwv", name=f"x1b{i}")
                   for i in range(2)]
            ys = [qkvp.tile([128, N], bf16, tag="qkv", name=f"ys{i}")
                  for i in range(3)]

            def tail_proj(t):
                sl = slice(t * TS, (t + 1) * TS)
                for ob in range(2):
                    pp = pq.tile([128, TS], f32, tag="pq", name="pp")
                    for kb in range(2):
                        nc.tensor.matmul(
                            pp,
                            lhsT=wproj_s[:, kb, ob * 128:(ob + 1) * 128],
                            rhs=attn_s[kb][:, sl],
                            start=(kb == 0), stop=(kb == 1))
                    nc.vector.tensor_tensor(
                        out=x1b[ob][:, sl], in0=pp, in1=xb_s[:, ob, sl],
                        op=OP.add)

            def tail_mlp1(t):
                sl = slice(t * TS, (t + 1) * TS)
                for mb in range(3):
                    rows = 128 if mb < 2 else HID - 256
                    pm = pdw.tile([128, TS], f32, tag="pdw", name="pm")
                    for kb in range(2):
                        nc.tensor.matmul(
                            pm[:rows, :],
                            lhsT=wm1_s[:, kb, mb * 128:mb * 128 + rows],
                            rhs=x1b[kb][:, sl],
                            start=(kb == 0), stop=(kb == 1))
                    nc.scalar.activation(
                        out=ys[mb][:rows, sl],
                        in_=pm[:rows, :], func=AF.Gelu_apprx_tanh,
                        bias=b1_s[:rows, mb:mb + 1])

            def tail_mlp2(t):
                sl = slice(t * TS, (t + 1) * TS)
                for ob in range(2):
                    pm2 = ptp.tile([128, TS], f32, tag="tp", name="pm2")
                    for kb in range(3):
                        rows = 128 if kb < 2 else HID - 256
                        nc.tensor.matmul(
                            pm2,
                            lhsT=wm2_s[:rows, kb, ob * 128:(ob + 1) * 128],
                            rhs=ys[kb][:rows, sl],
                            start=(kb == 0), stop=(kb == 2))
                    nc.vector.scalar_tensor_tensor(
                        out=xb_s[:, ob, sl], in0=pm2,
                        scalar=b2_s[:, ob:ob + 1], in1=x1b[ob][:, sl],
                        op0=OP.add, op1=OP.add)
                    nc.sync.dma_start(out=out_d[:, ob, sl], in_=xb_s[:, ob, sl])

            for t in range(11):
                if t < 8:
                    do_av_tile(0, t)
                    do_av_tile(1, t)
                if 1 <= t < 9:
                    tail_proj(t - 1)
                if 2 <= t < 10:
                    tail_mlp1(t - 2)
                if 3 <= t < 11:
                    tail_mlp2(t - 3)

    return nc


def _prep_shared(w_qkv, w_dw, temperature, w_proj, w_mlp1, b_mlp1, w_mlp2, b_mlp2):
    f32 = np.float32
    shared = {}
    shared["wqkvT"] = np.ascontiguousarray(
        w_qkv.T.reshape(2, 128, 3 * C).transpose(1, 0, 2)).astype(BF16)
    wd = np.zeros((128, 8, NB_QKV, 128), BF16)
    for ti, (dy, dx) in enumerate(PE_TAPS + V_EXTRA):
        for cb in range(NB_QKV):
            w = w_dw[cb * 128:(cb + 1) * 128, 0, dy + 1, dx + 1].astype(f32)
            wd[:, ti, cb, :] = np.diag(w).astype(BF16)
    shared["wdiag"] = wd
    wt = np.zeros((128, 12, 9), f32)
    for cb in range(NB_QKV):
        for t in range(9):
            wt[:, cb, t] = w_dw[cb * 128:(cb + 1) * 128, 0, t // 3, t % 3]
    wt[:, 6:, :] = -wt[:, :6, :]
    shared["wdw"] = wt
    shared["wprojT"] = np.ascontiguousarray(
        w_proj.T.reshape(2, 128, C).transpose(1, 0, 2)).astype(BF16)
    shared["wm1T"] = np.ascontiguousarray(
        w_mlp1.T.reshape(2, 128, HID).transpose(1, 0, 2)).astype(BF16)
    w2 = np.zeros((384, C), f32)
    w2[:HID] = w_mlp2.T
    shared["wm2T"] = np.ascontiguousarray(
        w2.reshape(3, 128, C).transpose(1, 0, 2)).astype(BF16)
    b1 = np.zeros((384,), f32)
    b1[:HID] = b_mlp1
    shared["b1"] = np.ascontiguousarray(b1.reshape(3, 128).T)
    shared["b2"] = np.ascontiguousarray(b_mlp2.astype(f32).reshape(2, 128).T)
    t = temperature.reshape(NH).astype(f32)
    tv = np.zeros((128, 2), f32)
    for g in range(2):
        tv[:, g] = np.repeat(t[g * 4:(g + 1) * 4], 32)
    shared["tempvec"] = tv
    return shared


def kernel(x, w_qkv, w_dw, temperature, w_proj, w_mlp1, b_mlp1, w_mlp2, b_mlp2,
           _trace=False):
    from concourse.bass_utils import run_bass_kernel_spmd

    if "nc" not in _CACHE:
        nc = _build_bass()
        nc.finalize()
        _CACHE["nc"] = nc
    nc = _CACHE["nc"]

    x = np.asarray(x, np.float32)
    B = x.shape[0]
    shared = _prep_shared(
        np.asarray(w_qkv, np.float32), np.asarray(w_dw, np.float32),
        np.asarray(temperature, np.float32), np.asarray(w_proj, np.float32),
        np.asarray(w_mlp1, np.float32), np.asarray(b_mlp1, np.float32),
        np.asarray(w_mlp2, np.float32), np.asarray(b_mlp2, np.float32))

    in_maps = []
    for i in range(B):
        m = dict(shared)
        xi = np.ascontiguousarray(x[i].reshape(2, 128, N).transpose(1, 0, 2))
        m["xb"] = xi.astype(BF16)
        in_maps.append(m)

    res = run_bass_kernel_spmd(nc, in_maps, core_ids=list(range(B)),
                               trace=_trace)
    outs = np.stack([
        np.asarray(r["out"]).astype(np.float32).transpose(1, 0, 2)
        .reshape(C, H, W) for r in res.results
    ])
    if _trace:
        _CACHE["last_exec_ns"] = res.exec_time_ns
        _CACHE["last_profile"] = res.profile_json
    return outs


# revision 36
# speedup vs baseline: 1.6615x; 1.0107x over previous
"""Trainium2 Bass kernel for nn_CustomABlock (MDTA transformer block).

Per-core layout: one batch image [C=256, N=4096(=64x64)] per NeuronCore,
data-parallel over B=8 across 8 cores, all params replicated.

dwconv 3x3 tap split (s = 64*dy + dx, flat shift over zero-padded qkv):
  q/k blocks: 6 PE diag-matmul taps (corners + (0,+-1)); DVE: (-1,0) folded
    into the PSUM-drain STT, (0,0) and (1,0) as TS(4x)+TT(2x) pairs.
  v blocks: 8 PE taps (adds (0,0),(1,0)); DVE only drains + fixes —
    frees DVE in the attention/tail transition.
  6 x-wrap column fixes per block (negated weights), all DVE work split
  per half-block so downstream av/tail unlock early.
Residuals in bf16 (x loaded bf16 only); softmax without the row-max pass
(|logit| <= t by Cauchy-Schwarz on normalized q,k); temperature/|q| folded
into the EXP scale; |k| scale applied once per k block; all four
sqrt/recips batched into one op each to minimize ACT table loads.
"""

import numpy as np
import ml_dtypes

BF16 = ml_dtypes.bfloat16

C = 256          # dim
N = 4096         # 64*64
H = W = 64
NH = 8           # heads
HID = 307        # mlp hidden
NB_QKV = 6       # qkv channel blocks of 128
TS = 512
PAD = 66         # zero pad elems each side of qkv (covers |s| <= 65, even)

# PE taps: flat full-range diag matmuls over the zero-padded qkv buffer.
PE_TAPS = [(0, -1), (0, 1), (-1, -1), (-1, 1), (1, -1), (1, 1)]
V_EXTRA = [(0, 0), (1, 0)]           # extra PE taps for v blocks
# x-wrap column fixes for PE taps: (tap, out_y0, out_y1, out_x, in_dy, in_x)
#   dw3[:, y0:y1, ox] -= w * qk3[:, y0+idy:y1+idy, ix]
FIXES = [
    ((0, -1), 1, 64, 0, -1, 63),
    ((-1, -1), 2, 64, 0, -2, 63),
    ((1, -1), 0, 64, 0, 0, 63),
    ((0, 1), 0, 63, 63, 1, 0),
    ((-1, 1), 0, 64, 63, 0, 0),
    ((1, 1), 0, 62, 63, 2, 0),
]

_CACHE = {}


def _build_bass():
    import os
    K_DMAADD = os.environ.get("KDMAADD", "0") == "1"
    import concourse.bass as bass
    from concourse import bacc
    from concourse import mybir
    from concourse.tile import TileContext
    from concourse.masks import make_identity

    dt = mybir.dt
    f32 = dt.float32
    bf16 = dt.bfloat16
    AF = mybir.ActivationFunctionType
    OP = mybir.AluOpType

    nc = bacc.Bacc("TRN2")

    # ---- DRAM I/O (per-core) ----
    xb_d = nc.dram_tensor("xb", [128, 2, N], bf16, kind="ExternalInput")
    wqkv_d = nc.dram_tensor("wqkvT", [128, 2, 3 * C], bf16, kind="ExternalInput")
    wdiag_d = nc.dram_tensor("wdiag", [128, 6, NB_QKV, 128], bf16,
                             kind="ExternalInput")
    # wdw[:, ob, t]: 9 taps fp32 per block; wdw[:, 6+ob, t]: negated
    wdw_d = nc.dram_tensor("wdw", [128, 12, 9], f32, kind="ExternalInput")
    wproj_d = nc.dram_tensor("wprojT", [128, 2, C], bf16, kind="ExternalInput")
    wm1_d = nc.dram_tensor("wm1T", [128, 2, 384], bf16, kind="ExternalInput")
    wm2_d = nc.dram_tensor("wm2T", [128, 3, C], bf16, kind="ExternalInput")
    b1_d = nc.dram_tensor("b1", [128, 3], f32, kind="ExternalInput")
    b2_d = nc.dram_tensor("b2", [128, 2], f32, kind="ExternalInput")
    tv_d = nc.dram_tensor("tempvec", [128, 2], f32, kind="ExternalInput")
    out_d = nc.dram_tensor("out", [128, 2, N], bf16, kind="ExternalOutput")

    with TileContext(nc) as tc:
        with (
            tc.tile_pool(name="wpool", bufs=1) as wpool,
            tc.tile_pool(name="xpool", bufs=1) as xpool,
            tc.tile_pool(name="qkvp", bufs=3) as qkvp,      # qkv blocks / ys reuse
            tc.tile_pool(name="dwqk", bufs=4) as dwqk_p,    # dw q/k blocks / attn_s
            tc.tile_pool(name="dwv", bufs=4) as dwv_p,      # dw v blocks / x1b reuse
            tc.tile_pool(name="qt", bufs=1) as qt_p,
            tc.tile_pool(name="scr", bufs=2) as scr_p,
            tc.tile_pool(name="small", bufs=12) as small_p,
            tc.tile_pool(name="apool", bufs=2) as a_p,
            tc.tile_pool(name="pq", bufs=4, space="PSUM") as pq,      # [128,512] f32
            tc.tile_pool(name="pdw", bufs=2, space="PSUM") as pdw,    # [128,512] f32
            tc.tile_pool(name="ptp", bufs=2, space="PSUM") as ptp,    # [128,1024] bf16
        ):
            # ---- load x & weights (critical path first) ----
            xb_s = xpool.tile([128, 2, N], bf16)
            wqkv_s = wpool.tile([128, 2, 3 * C], bf16)
            for kb in range(2):
                nc.sync.dma_start(
                    out=xb_s[:, kb, 0:2048], in_=xb_d[:, kb, 0:2048])
            for kb in range(2):
                nc.sync.dma_start(out=wqkv_s[:, kb, :], in_=wqkv_d[:, kb, :])
            wdiag_s = wpool.tile([128, 6, NB_QKV, 128], bf16)
            nc.sync.dma_start(out=wdiag_s, in_=wdiag_d[:, :, :, :])
            for kb in range(2):
                nc.sync.dma_start(
                    out=xb_s[:, kb, 2048:4096], in_=xb_d[:, kb, 2048:4096])
            wdw_s = wpool.tile([128, 12, 9], f32)
            nc.sync.dma_start(out=wdw_s, in_=wdw_d[:, :, :])
            wproj_s = wpool.tile([128, 2, C], bf16)
            nc.sync.dma_start(out=wproj_s, in_=wproj_d[:, :, :])
            wm1_s = wpool.tile([128, 2, 384], bf16)
            nc.sync.dma_start(out=wm1_s, in_=wm1_d[:, :, :])
            wm2_s = wpool.tile([128, 3, C], bf16)
            nc.sync.dma_start(out=wm2_s, in_=wm2_d[:, :, :])
            b1_s = wpool.tile([128, 3], f32)
            nc.sync.dma_start(out=b1_s, in_=b1_d[:, :])
            b2_s = wpool.tile([128, 2], f32)
            nc.sync.dma_start(out=b2_s, in_=b2_d[:, :])
            tv_s = wpool.tile([128, 2], f32)
            nc.sync.dma_start(out=tv_s, in_=tv_d[:, :])

            ident = wpool.tile([128, 128], bf16)
            make_identity(nc, ident)

            dw_tiles = [None] * NB_QKV
            qT_s = qt_p.tile([128, 32, C], bf16, tag="qT")
            kT_s = qt_p.tile([128, 32, C], bf16, tag="kT")
            attn_s = [dwqk_p.tile([128, N], bf16, tag="dwqk", name=f"attn{g}")
                      for g in range(2)]
            nrm2 = small_p.tile([128, 4], f32, tag="nrm2")
            rnrm = small_p.tile([128, 4], f32, tag="rnrm")
            snrm = small_p.tile([128, 4], f32, tag="snrm")
            qsc = small_p.tile([128, 2], f32, tag="qsc")
            rs_v = [None, None]
            At_v = [None, None]

            def wap(ob, dy, dx, neg=False):
                t = (dy + 1) * 3 + (dx + 1)
                o = 6 + ob if neg else ob
                return wdw_s[:, o, t:t + 1]

            def do_block(ob):
                # qkv = W_qkv @ x -> PSUM [128,512] -> bf16 SBUF (ACT drain)
                qkvz = qkvp.tile([128, N + 2 * PAD], bf16, tag="qkv",
                                 name=f"qkv{ob}")
                nc.vector.memset(qkvz[:, 0:PAD], 0.0)
                nc.vector.memset(qkvz[:, PAD + N:], 0.0)
                qkv_t = qkvz[:, PAD:PAD + N]
                for t in range(8):
                    ps = pq.tile([128, TS], f32, tag="pq", name="ps")
                    for kb in range(2):
                        nc.tensor.matmul(
                            ps,
                            lhsT=wqkv_s[:, kb, ob * 128:(ob + 1) * 128],
                            rhs=xb_s[:, kb, t * TS:(t + 1) * TS],
                            start=(kb == 0), stop=(kb == 1),
                        )
                    nc.scalar.copy(out=qkv_t[:, t * TS:(t + 1) * TS], in_=ps)

                dw_t = (dwqk_p if ob < 4 else dwv_p).tile(
                    [128, N], bf16, tag=("dwqk" if ob < 4 else "dwv"),
                    name=f"dw{ob}")
                dw_tiles[ob] = dw_t
                dw3 = dw_t.rearrange("p (y x) -> p y x", y=H)
                qk3 = qkv_t.rearrange("p (y x) -> p y x", y=H)

                taps = PE_TAPS
                wm10 = wap(ob, -1, 0)
                for half in range(2):
                    for t8 in range(half * 4, half * 4 + 4):
                        pd = pdw.tile([128, TS], f32, tag="pdw", name="pd")
                        c0 = t8 * TS
                        for j, (dy, dx) in enumerate(taps):
                            s = dy * 64 + dx
                            nc.tensor.matmul(
                                pd,
                                lhsT=wdiag_s[:, j, ob, :],
                                rhs=qkvz[:, PAD + c0 + s:PAD + c0 + s + TS],
                                start=(j == 0), stop=(j == len(taps) - 1),
                            )
                        if t8 == 0:
                            # row 0 has no (-1,0) tap: plain drain on ACT
                            nc.scalar.copy(out=dw_t[:, 0:64], in_=pd[:, 0:64])
                            nc.vector.scalar_tensor_tensor(
                                out=dw_t[:, 64:TS], in0=qkv_t[:, 0:TS - 64],
                                scalar=wm10, in1=pd[:, 64:TS],
                                op0=OP.mult, op1=OP.add)
                        else:
                            nc.vector.scalar_tensor_tensor(
                                out=dw_t[:, c0:c0 + TS],
                                in0=qkv_t[:, c0 - 64:c0 + TS - 64],
                                scalar=wm10, in1=pd,
                                op0=OP.mult, op1=OP.add)
                    h0 = half * 2048
                    h1 = h0 + 2048
                    if True:
                        # taps (0,0) and (1,0) as TS(4x) + TT(2x) per half
                        sc = scr_p.tile([128, 2048], bf16, tag="ts",
                                        name=f"ts{ob}_{half}_0")
                        nc.vector.tensor_scalar_mul(
                            sc, qkv_t[:, h0:h1], wap(ob, 0, 0))
                        e1 = min(h1, N - 64)
                        sc2 = scr_p.tile([128, 2048], bf16, tag="ts",
                                         name=f"ts{ob}_{half}_1")
                        nc.vector.tensor_scalar_mul(
                            sc2[:, 0:e1 - h0], qkv_t[:, h0 + 64:e1 + 64],
                            wap(ob, 1, 0))
                        if K_DMAADD:
                            nc.gpsimd.dma_start(
                                out=dw_t[:, h0:h1], in_=sc, accum_op=OP.add)
                            nc.gpsimd.dma_start(
                                out=dw_t[:, h0:e1], in_=sc2[:, 0:e1 - h0],
                                accum_op=OP.add)
                        else:
                            nc.vector.tensor_tensor(
                                out=dw_t[:, h0:h1], in0=dw_t[:, h0:h1],
                                in1=sc, op=OP.add)
                            nc.vector.tensor_tensor(
                                out=dw_t[:, h0:e1], in0=dw_t[:, h0:e1],
                                in1=sc2[:, 0:e1 - h0], op=OP.add)
                    # x-wrap column fixes for this half (negated weights)
                    yh0, yh1 = half * 32, half * 32 + 32
                    for (dy, dx), y0, y1, ox, idy, ix in FIXES:
                        ya, yb = max(y0, yh0), min(y1, yh1)
                        if ya < yb:
                            nc.vector.scalar_tensor_tensor(
                                out=dw3[:, ya:yb, ox:ox + 1],
                                in0=qk3[:, ya + idy:yb + idy, ix:ix + 1],
                                scalar=wap(ob, dy, dx, neg=True),
                                in1=dw3[:, ya:yb, ox:ox + 1],
                                op0=OP.mult, op1=OP.add)

                if ob < 4:
                    # sum of squares (ACT Square + accumulator)
                    sq = scr_p.tile([128, N], bf16, tag="sq")
                    nc.scalar.activation(
                        out=sq, in_=dw_t, func=AF.Square,
                        accum_out=nrm2[:, ob:ob + 1])
                if ob < 2:
                    do_transpose(ob)

            def do_transpose(ob):
                dw_t = dw_tiles[ob]
                dst = qT_s if ob < 2 else kT_s
                cof = (ob % 2) * 128
                for g in range(4):
                    tp_t = ptp.tile([128, 1024], bf16, tag="tp")
                    for i in range(8):
                        nb = g * 8 + i
                        nc.tensor.transpose(
                            tp_t[:, i * 128:(i + 1) * 128],
                            dw_t[:, nb * 128:(nb + 1) * 128], ident)
                    nc.vector.tensor_copy(
                        out=dst[:, g * 8:g * 8 + 8, cof:cof + 128],
                        in_=tp_t.rearrange("p (a b) -> p a b", a=8))

            def do_norms():
                # batched sqrt/recip for all four q/k norms; k-row scales
                nc.scalar.activation(out=snrm, in_=nrm2, func=AF.Sqrt)
                nc.vector.reciprocal(rnrm, snrm)
                nc.vector.tensor_mul(qsc, rnrm[:, 0:2], tv_s)
                for ob in (2, 3):
                    nc.vector.tensor_scalar_mul(
                        dw_tiles[ob], dw_tiles[ob], rnrm[:, ob:ob + 1])
                    do_transpose(ob)

            def do_gram(g):
                pgt = pdw.tile([128, TS], f32, tag="pdw", name=f"pg{g}")
                pg = pgt[:, 0:128]
                co = g * 128
                for nb in range(32):
                    nc.tensor.matmul(
                        pg,
                        lhsT=qT_s[:, nb, co:co + 128],
                        rhs=kT_s[:, nb, co:co + 128],
                        start=(nb == 0), stop=(nb == 31),
                    )
                A_t = a_p.tile([128, 128], bf16, tag="A")
                nc.vector.memset(A_t, 0.0)
                sm = small_p.tile([128, 1], f32, tag="sm")
                for h in range(4):
                    r0, r1 = h * 32, h * 32 + 32
                    nc.scalar.activation(
                        out=A_t[r0:r1, r0:r1], in_=pg[r0:r1, r0:r1],
                        func=AF.Exp, scale=qsc[r0:r1, g:g + 1],
                        accum_out=sm[r0:r1, :])
                rs = small_p.tile([128, 1], f32, tag="rs", name=f"rs{g}")
                nc.vector.reciprocal(rs, sm)
                rs_v[g] = rs
                pa_t = ptp.tile([128, 1024], bf16, tag="tp")
                pa = pa_t[:, 0:128]
                nc.tensor.transpose(pa, A_t, ident)
                At = a_p.tile([128, 128], bf16, tag="At", name=f"At{g}")
                nc.vector.tensor_copy(out=At, in_=pa)
                At_v[g] = At

            def do_av_tile(g, t):
                pv = pq.tile([128, TS], f32, tag="pq", name="pv")
                nc.tensor.matmul(
                    pv, lhsT=At_v[g],
                    rhs=dw_tiles[4 + g][:, t * TS:(t + 1) * TS],
                    start=True, stop=True)
                nc.scalar.mul(attn_s[g][:, t * TS:(t + 1) * TS],
                              pv, rs_v[g])

            do_block(0)
            do_block(1)
            do_block(2)
            do_block(3)
            do_block(4)
            do_norms()
            do_gram(0)
            do_gram(1)
            do_block(5)

            # ---- software-pipelined tail: av -> proj -> mlp1 -> mlp2 ----
            x1b = [dwv_p.tile([128, N], bf16, tag="dwv", name=f"x1b{i}")
                   for i in range(2)]
            ys = [qkvp.tile([128, N], bf16, tag="qkv", name=f"ys{i}")
                  for i in range(3)]

            def tail_proj(t):
                sl = slice(t * TS, (t + 1) * TS)
                for ob in range(2):
                    pp = pq.tile([128, TS], f32, tag="pq", name="pp")
                    for kb in range(2):
                        nc.tensor.matmul(
                            pp,
                            lhsT=wproj_s[:, kb, ob * 128:(ob + 1) * 128],
                            rhs=attn_s[kb][:, sl],
                            start=(kb == 0), stop=(kb == 1))
                    nc.vector.tensor_tensor(
                        out=x1b[ob][:, sl], in0=pp, in1=xb_s[:, ob, sl],
                        op=OP.add)

            def tail_mlp1(t):
                sl = slice(t * TS, (t + 1) * TS)
                for mb in range(3):
                    pm = pdw.tile([128, TS], f32, tag="pdw", name="pm")
                    for kb in range(2):
                        nc.tensor.matmul(
                            pm,
                            lhsT=wm1_s[:, kb, mb * 128:(mb + 1) * 128],
                            rhs=x1b[kb][:, sl],
                            start=(kb == 0), stop=(kb == 1))
                    nc.scalar.activation(
                        out=ys[mb][:, sl],
                        in_=pm, func=AF.Gelu_apprx_tanh,
                        bias=b1_s[:, mb:mb + 1])

            def tail_mlp2(t):
                sl = slice(t * TS, (t + 1) * TS)
                for ob in range(2):
                    pm2 = ptp.tile([128, TS], f32, tag="tp", name="pm2")
                    for kb in range(3):
                        nc.tensor.matmul(
                            pm2,
                            lhsT=wm2_s[:, kb, ob * 128:(ob + 1) * 128],
                            rhs=ys[kb][:, sl],
                            start=(kb == 0), stop=(kb == 2))
                    nc.vector.scalar_tensor_tensor(
                        out=xb_s[:, ob, sl], in0=pm2,
                        scalar=b2_s[:, ob:ob + 1], in1=x1b[ob][:, sl],
                        op0=OP.add, op1=OP.add)
                    nc.sync.dma_start(out=out_d[:, ob, sl], in_=xb_s[:, ob, sl])

            for t in range(11):
                if t < 8:
                    do_av_tile(0, t)
                    do_av_tile(1, t)
                if 1 <= t < 9:
                    tail_proj(t - 1)
                if 2 <= t < 10:
                    tail_mlp1(t - 2)
                if 3 <= t < 11:
                    tail_mlp2(t - 3)

    return nc


def _prep_shared(w_qkv, w_dw, temperature, w_proj, w_mlp1, b_mlp1, w_mlp2, b_mlp2):
    f32 = np.float32
    shared = {}
    shared["wqkvT"] = np.ascontiguousarray(
        w_qkv.T.reshape(2, 128, 3 * C).transpose(1, 0, 2)).astype(BF16)
    wd = np.zeros((128, 6, NB_QKV, 128), BF16)
    for ti, (dy, dx) in enumerate(PE_TAPS):
        for cb in range(NB_QKV):
            w = w_dw[cb * 128:(cb + 1) * 128, 0, dy + 1, dx + 1].astype(f32)
            wd[:, ti, cb, :] = np.diag(w).astype(BF16)
    shared["wdiag"] = wd
    wt = np.zeros((128, 12, 9), f32)
    for cb in range(NB_QKV):
        for t in range(9):
            wt[:, cb, t] = w_dw[cb * 128:(cb + 1) * 128, 0, t // 3, t % 3]
    wt[:, 6:, :] = -wt[:, :6, :]
    shared["wdw"] = wt
    shared["wprojT"] = np.ascontiguousarray(
        w_proj.T.reshape(2, 128, C).transpose(1, 0, 2)).astype(BF16)
    w1 = np.zeros((256, 384), f32)
    w1[:, :HID] = w_mlp1.T
    shared["wm1T"] = np.ascontiguousarray(
        w1.reshape(2, 128, 384).transpose(1, 0, 2)).astype(BF16)
    w2 = np.zeros((384, C), f32)
    w2[:HID] = w_mlp2.T
    shared["wm2T"] = np.ascontiguousarray(
        w2.reshape(3, 128, C).transpose(1, 0, 2)).astype(BF16)
    b1 = np.zeros((384,), f32)
    b1[:HID] = b_mlp1
    shared["b1"] = np.ascontiguousarray(b1.reshape(3, 128).T)
    shared["b2"] = np.ascontiguousarray(b_mlp2.astype(f32).reshape(2, 128).T)
    t = temperature.reshape(NH).astype(f32)
    tv = np.zeros((128, 2), f32)
    for g in range(2):
        tv[:, g] = np.repeat(t[g * 4:(g + 1) * 4], 32)
    shared["tempvec"] = tv
    return shared


def kernel(x, w_qkv, w_dw, temperature, w_proj, w_mlp1, b_mlp1, w_mlp2, b_mlp2,
           _trace=False):
    from concourse.bass_utils import run_bass_kernel_spmd

    if "nc" not in _CACHE:
        nc = _build_bass()
        nc.finalize()
        _CACHE["nc"] = nc
    nc = _CACHE["nc"]

    x = np.asarray(x, np.float32)
    B = x.shape[0]
    shared = _prep_shared(
        np.asarray(w_qkv, np.float32), np.asarray(w_dw, np.float32),
        np.asarray(temperature, np.float32), np.asarray(w_proj, np.float32),
        np.asarray(w_mlp1, np.float32), np.asarray(b_mlp1, np.float32),
        np.asarray(w_mlp2, np.float32), np.asarray(b_mlp2, np.float32))

    in_maps = []
    for i in range(B):
        m = dict(shared)
        xi = np.ascontiguousarray(x[i].reshape(2, 128, N).transpose(1, 0, 2))
        m["xb"] = xi.astype(BF16)
        in_maps.append(m)

    res = run_bass_kernel_spmd(nc, in_maps, core_ids=list(range(B)),
                               trace=_trace)
    outs = np.stack([
        np.asarray(r["out"]).astype(np.float32).transpose(1, 0, 2)
        .reshape(C, H, W) for r in res.results
    ])
    if _trace:
        _CACHE["last_exec_ns"] = res.exec_time_ns
        _CACHE["last_profile"] = res.profile_json
    return outs
